# revision 8
# baseline (speedup 1.0000x reference)
"""Trainium2 Bass kernel for a dense transformer AttentionBlock (optimized v2).

Problem: x[2,2048,1024] fp32 -> LN1 -> MHA(16 heads, hd=64) + residual
         -> LN2 -> FFN(4096, relu) + residual.

Sharding: 8-way data parallel. Core c handles batch b=c//4 and query chunk
qc=c%4 (512 tokens). K/V are computed for the full 2048-token sequence on
each core (redundant within a batch group, but no collectives needed).
The host rotates each core's token axis so its query chunk is rows 0:512 —
softmax over keys is permutation invariant so key order does not matter.

Host-side exact weight algebra (one-time O(H^2) prep, keeps device math
identical): LN affines are folded into the projection weights
(Wq' = g1*Wq rows, bq' = bq + b1@Wq, same for K/V; W1' = g2*W1,
bf1' = bf1 + b2@W1) and the V bias is folded through the output projection
(bo2 = bo + bv'@Wo, exact because softmax rows sum to 1). Wq/Wk/Wv ship
pre-cast to bf16 (the attention matmul dtype) to halve their DMA bytes.

Precision: FFN / output-proj matmuls in float32r (full PE rate at N>=512);
attention datapath bf16 with fp32 PSUM. Softmax uses exp without max
subtraction; denominator accumulated via a ones-column in the V operand and
divided out per head (PE broadcast of the reciprocal row).

Scheduling notes (tuned against the TimelineSim cost model): all DMA
transfers serialize on one ~360 GB/s pipe, so x tiles are dispatched ahead
of Wv; Wo / x-residual / first W1 tiles live in pools opened before phase 1
so their SP-queue DMAs (emitted after phase 1) stream during attention; LN
runs two tiles ahead of the transpose+V work so PE never idles (idle gaps
cost 2-3.7x p-state matmul penalties for the next ~3us).
"""

import numpy as np
from contextlib import ExitStack

B, S, H = 2, 2048, 1024
NH, HD = 16, 64
FF = 4 * H
EPS = 1e-5
P = 128
SQ = 512          # query-chunk tokens per core
N_CORES = 8
TT_Q = SQ // P    # 4 token tiles in the query chunk
TT_S = S // P     # 16 token tiles in the full sequence
KH = H // P       # 8 k-tiles over hidden dim


def _build_nc():
    import concourse.bass as bass
    import concourse.mybir as mybir
    import concourse.tile as tile
    from concourse import bacc
    from concourse.masks import make_identity

    dt = mybir.dt
    f32 = dt.float32
    f32r = dt.float32r
    bf16 = dt.bfloat16
    AF = mybir.ActivationFunctionType
    ALU = mybir.AluOpType

    nc = bacc.Bacc(None, target_bir_lowering=False)

    xp = nc.dram_tensor("xp", [S, H], f32, kind="ExternalInput")
    Wq = nc.dram_tensor("Wq", [H, H], bf16, kind="ExternalInput")
    Wk = nc.dram_tensor("Wk", [H, H], bf16, kind="ExternalInput")
    Wv = nc.dram_tensor("Wv", [H, H], bf16, kind="ExternalInput")
    Wo = nc.dram_tensor("Wo", [H, H], f32, kind="ExternalInput")
    W1 = nc.dram_tensor("W1", [H, FF], f32, kind="ExternalInput")
    W2 = nc.dram_tensor("W2", [FF, H], f32, kind="ExternalInput")
    bq = nc.dram_tensor("bq", [H], f32, kind="ExternalInput")
    bk = nc.dram_tensor("bk", [H], f32, kind="ExternalInput")
    bo2 = nc.dram_tensor("bo2", [H], f32, kind="ExternalInput")
    bf1 = nc.dram_tensor("bf1", [FF], f32, kind="ExternalInput")
    bf2 = nc.dram_tensor("bf2", [H], f32, kind="ExternalInput")
    y = nc.dram_tensor("y", [SQ, H], f32, kind="ExternalOutput")

    def col_view(dram_vec, n):
        # DRAM [n*P] viewed as [P, n]: element [p, j] = vec[j*P + p]
        return dram_vec.rearrange("(a p) -> p a", p=P)

    def layernorm_tile(pool, src_ap, tag, xn_dt=f32):
        """token-major LN of one [P, H] tile -> (x - m) * rstd. The per
        feature g/b affine is folded into the weights on the host."""
        stats = pool.tile([P, 2, 6], f32, tag=tag + "st")
        nc.vector.bn_stats(out=stats[:, 0, :], in_=src_ap[:, 0:512])
        nc.vector.bn_stats(out=stats[:, 1, :], in_=src_ap[:, 512:1024])
        mv = pool.tile([P, 2], f32, tag=tag + "mv")
        nc.vector.bn_aggr(out=mv, in_=stats)
        rstd = pool.tile([P, 1], f32, tag=tag + "rs")
        nc.scalar.activation(out=rstd, in_=mv[:, 1:2], func=AF.Sqrt,
                             bias=eps_t, scale=1.0)
        nc.vector.reciprocal(out=rstd, in_=rstd)
        negmr = pool.tile([P, 1], f32, tag=tag + "nm")
        nc.vector.scalar_tensor_tensor(
            out=negmr, in0=mv[:, 0:1], scalar=rstd, in1=negone,
            op0=ALU.mult, op1=ALU.mult)
        xn = pool.tile([P, H], xn_dt, tag=tag + "xn")
        nc.scalar.activation(out=xn, in_=src_ap, func=AF.Identity,
                             bias=negmr, scale=rstd)
        return xn

    with tile.TileContext(nc) as tc:
        with (
            tc.tile_pool(name="consts", bufs=1) as consts,
            tc.tile_pool(name="persistA", bufs=1) as persistA,
        ):
            ctxT = persistA.tile([P, KH, SQ], f32r)      # 16 KB/part

            ident_b = consts.tile([P, P], bf16)
            make_identity(nc, ident_b)
            eps_t = consts.tile([P, 1], f32)
            nc.vector.memset(eps_t, EPS)
            negone = consts.tile([P, 1], f32)
            nc.vector.memset(negone, -1.0)
            zsrc = consts.tile([P, HD], f32)
            nc.vector.memset(zsrc, 0.0)
            # f32r constant 1.0 (memset can't emit f32r; ACT Copy(0*x+1) can)
            ones_t = consts.tile([1, HD], f32r)
            nc.scalar.activation(out=ones_t, in_=zsrc[0:1, :], func=AF.Copy,
                                 bias=1.0, scale=0.0)
            ident_f = consts.tile([P, P], f32)
            make_identity(nc, ident_f)
            ident_r = consts.tile([P, P], f32r)
            nc.scalar.copy(out=ident_r, in_=ident_f)

            # persistB (phase-3+ accumulators) and the prefetch pools open
            # BEFORE phase 1 so they get SBUF space disjoint from the
            # phase-1/2 pools: their SP-queue DMAs (emitted after the
            # phase-1 loop) then stream during attention with no space-reuse
            # dependencies. prefetch_es closes right after phase 3 (LIFO) to
            # make room for the FFN pools.
            persistB_es = ExitStack()
            persistB = persistB_es.enter_context(tc.tile_pool(name="persistB", bufs=1))
            out_res = persistB.tile([P, TT_Q, H], f32)   # 16 KB/part
            stats2 = persistB.tile([P, TT_Q, KH, 6], f32)

            w1p_es = ExitStack()
            w1p = w1p_es.enter_context(tc.tile_pool(name="w1p", bufs=4))

            prefetch_es = ExitStack()
            wop = prefetch_es.enter_context(tc.tile_pool(name="wo", bufs=1))
            wqkv = prefetch_es.enter_context(tc.tile_pool(name="wqkv", bufs=2))
            qtp = prefetch_es.enter_context(tc.tile_pool(name="qt", bufs=2))

            with tc.tile_pool(name="xnTp", bufs=1) as xnTp:
                xnTc = [xnTp.tile([P, KH, SQ], bf16, tag=f"xnt{c}", name=f"xnt{c}")
                        for c in range(4)]               # 4 x 8 KB/part

                with tc.tile_pool(name="vap", bufs=1) as vap:
                    # token-major V (+ ones column for softmax denominator)
                    va = vap.tile([P, TT_S, NH, HD + 1], bf16)
                    nc.vector.memset(va[:, :, :, HD:HD + 1], 1.0)

                    # ------- Phase 1: LN1 + transpose + V projection -------
                    with (
                        tc.tile_pool(name="ln", bufs=4) as ln,
                        tc.tile_pool(name="lnst", bufs=4) as lnst,
                        tc.tile_pool(name="wvp", bufs=1) as wvp,
                        tc.tile_pool(name="ps_t", bufs=2, space="PSUM") as ps_t,
                        tc.tile_pool(name="ps_v", bufs=2, space="PSUM") as ps_v,
                    ):
                        # LN is software-pipelined two tiles ahead of the
                        # transpose/V work so PE never waits on the LN chain.
                        xns = {}

                        def emit_ln(i):
                            xt = ln.tile([P, H], f32, tag="xt", name="xt")
                            nc.sync.dma_start(out=xt, in_=xp[i * P:(i + 1) * P, :])
                            xns[i] = layernorm_tile(lnst, xt, "l1", xn_dt=bf16)

                        emit_ln(0)
                        emit_ln(1)
                        emit_ln(2)

                        # Wv quarters queue behind the first x tiles on the
                        # (serial) DMA pipe.
                        wv = wvp.tile([P, KH, H], bf16)
                        wv_view = Wv.rearrange("(a p) c -> p a c", p=P)
                        nc.scalar.dma_start(out=wv[:, :, 0:256], in_=wv_view[:, :, 0:256])
                        nc.gpsimd.dma_start(out=wv[:, :, 256:512], in_=wv_view[:, :, 256:512])
                        nc.scalar.dma_start(out=wv[:, :, 512:768], in_=wv_view[:, :, 512:768])
                        nc.gpsimd.dma_start(out=wv[:, :, 768:1024], in_=wv_view[:, :, 768:1024])

                        bq_t = consts.tile([P, KH], f32)
                        bk_t = consts.tile([P, KH], f32)
                        bo2_t = consts.tile([P, KH], f32)
                        bf2_t = consts.tile([P, KH], f32)
                        bf1_t = consts.tile([P, FF // P], f32)
                        nc.gpsimd.dma_start(out=bq_t, in_=col_view(bq, KH))
                        nc.gpsimd.dma_start(out=bk_t, in_=col_view(bk, KH))
                        nc.gpsimd.dma_start(out=bo2_t, in_=col_view(bo2, KH))
                        nc.gpsimd.dma_start(out=bf2_t, in_=col_view(bf2, KH))
                        nc.gpsimd.dma_start(out=bf1_t, in_=col_view(bf1, FF // P))

                        def emit_v(i):
                            # V projection for tile i (runs one tile behind
                            # the transposes so PE never waits on the
                            # PSUM->SBUF evacuation of xnT)
                            for fg in range(2):
                                psw = ps_v.tile([P, SQ], f32, tag="psv",
                                                name="psw")
                                for ks in range(KH):
                                    nc.tensor.matmul(
                                        psw, xnTc[i // 4][:, ks, (i % 4) * P:(i % 4 + 1) * P],
                                        wv[:, ks, fg * SQ:(fg + 1) * SQ],
                                        start=(ks == 0), stop=(ks == KH - 1))
                                dstv = va[:, i, 8 * fg:8 * fg + 8, 0:HD]
                                src = psw.rearrange("p (h d) -> p h d", d=HD)
                                if fg == 0:
                                    nc.vector.tensor_copy(out=dstv, in_=src)
                                else:
                                    nc.scalar.copy(out=dstv, in_=src)

                        def emit_q0():
                            # Q projection for head-group 0 (only needs
                            # xnTc[0], ready after tile 3) — fills the PE
                            # starvation window while LN1 still paces tiles.
                            wk4 = wqkv.tile([P, KH, 256], bf16, tag="wk4",
                                            name="wk4")
                            wq4 = wqkv.tile([P, KH, 256], bf16, tag="wq4",
                                            name="wq4")
                            nc.gpsimd.dma_start(
                                out=wk4,
                                in_=Wk.rearrange("(a p) c -> p a c", p=P)[:, :, 0:256])
                            nc.gpsimd.dma_start(
                                out=wq4,
                                in_=Wq.rearrange("(a p) c -> p a c", p=P)[:, :, 0:256])
                            QTp = []
                            for pair in range(2):
                                ps = ps_v.tile([P, SQ], f32, tag="psv",
                                               name="psq0")
                                for ks in range(KH):
                                    nc.tensor.matmul(
                                        ps, wq4[:, ks, pair * P:(pair + 1) * P],
                                        xnTc[0][:, ks, :],
                                        start=(ks == 0), stop=(ks == KH - 1))
                                qt0 = qtp.tile([P, SQ], bf16, tag=f"qtp{pair}",
                                               name=f"qtp{pair}")
                                nc.vector.tensor_scalar_add(
                                    out=qt0, in0=ps, scalar1=bq_t[:, pair:pair + 1])
                                QTp.append(qt0)
                            return wk4, wq4, QTp

                        g0_hoist = {}
                        for i in range(TT_S):
                            if i + 3 < TT_S:
                                emit_ln(i + 3)
                            if i == 5:
                                g0_hoist["v"] = emit_q0()
                            xn = xns.pop(i)
                            for j2 in range(2):
                                ps = ps_t.tile([P, 4, P], bf16, tag="pst")
                                for k in range(4):
                                    j = 4 * j2 + k
                                    nc.tensor.transpose(
                                        ps[:, k, :], xn[:, j * P:(j + 1) * P], ident_b)
                                dst = xnTc[i // 4][:, 4 * j2:4 * j2 + 4,
                                                  (i % 4) * P:(i % 4 + 1) * P]
                                if j2 == 0:
                                    nc.vector.tensor_copy(out=dst, in_=ps)
                                else:
                                    nc.scalar.copy(out=dst, in_=ps)
                            if i >= 1:
                                emit_v(i - 1)
                        emit_v(TT_S - 1)

                    # Prefetch DMAs for phase 3+ (SP queue drains these
                    # during attention; pools were opened before phase 1).
                    wo_ts = []
                    for os_ in range(KH):
                        # wo7 reuses wo0's buffer: its DMA waits (WAR) until
                        # the os_=0 matmuls of phase 3 release it, which is
                        # still well before os_=7 runs.
                        tagi = os_ % 7
                        wo_t = wop.tile([P, KH, P], f32r, tag=f"wo{tagi}",
                                        name=f"wo{tagi}")
                        nc.sync.dma_start(
                            out=wo_t,
                            in_=Wo.rearrange("(a p) c -> p a c", p=P)[:, :, os_ * P:(os_ + 1) * P].bitcast(f32r))
                        wo_ts.append(wo_t)
                    w1_ts = {}
                    for ft in range(4):
                        w1_t = w1p.tile([P, KH, P], f32r, tag="w1_t", name="w1_t")
                        nc.sync.dma_start(
                            out=w1_t,
                            in_=W1.rearrange("(a p) c -> p a c", p=P)[:, :, ft * P:(ft + 1) * P].bitcast(f32r))
                        w1_ts[ft] = w1_t

                    # ---------------- Phase 2: K/Q + attention ----------------
                    with ExitStack() as es2:
                        ktp = es2.enter_context(tc.tile_pool(name="kt", bufs=2))
                        pexp = es2.enter_context(tc.tile_pool(name="pexp", bufs=3))
                        attn_sm = es2.enter_context(tc.tile_pool(name="attn_sm", bufs=4))
                        ps_qkv = es2.enter_context(tc.tile_pool(name="ps_qkv", bufs=2, space="PSUM"))
                        ps_s = es2.enter_context(tc.tile_pool(name="ps_s", bufs=3, space="PSUM"))
                        ps_ctx = es2.enter_context(tc.tile_pool(name="ps_ctx", bufs=1, space="PSUM"))
                        ps_b = es2.enter_context(tc.tile_pool(name="ps_b", bufs=1, space="PSUM"))
                        for g4 in range(4):
                            c0 = g4 * 4 * HD  # first feature column of this group
                            if g4 == 0:
                                wk4, wq4, QTp = g0_hoist.pop("v")
                            else:
                                wk4 = wqkv.tile([P, KH, 256], bf16, tag="wk4")
                                wq4 = wqkv.tile([P, KH, 256], bf16, tag="wq4")
                                nc.gpsimd.dma_start(
                                    out=wk4,
                                    in_=Wk.rearrange("(a p) c -> p a c", p=P)[:, :, c0:c0 + 256])
                                nc.gpsimd.dma_start(
                                    out=wq4,
                                    in_=Wq.rearrange("(a p) c -> p a c", p=P)[:, :, c0:c0 + 256])

                                # Q^T pair-stacked: [128(2 heads), SQ] per pair
                                QTp = [qtp.tile([P, SQ], bf16, tag=f"qtp{pair}",
                                                name=f"qtp{pair}") for pair in range(2)]
                                for pair in range(2):
                                    hp = 2 * g4 + pair
                                    ps = ps_qkv.tile([P, SQ], f32, tag="psqkv")
                                    for ks in range(KH):
                                        nc.tensor.matmul(
                                            ps, wq4[:, ks, pair * P:(pair + 1) * P],
                                            xnTc[0][:, ks, :],
                                            start=(ks == 0), stop=(ks == KH - 1))
                                    nc.vector.tensor_scalar_add(
                                        out=QTp[pair], in0=ps,
                                        scalar1=bq_t[:, hp:hp + 1])

                            for pair in range(2):
                                hp = 2 * g4 + pair
                                # K^T pair-stacked: [128(2 heads), S]
                                KTp = ktp.tile([P, S], bf16, tag="ktp")
                                for t4 in range(4):
                                    ps = ps_qkv.tile([P, SQ], f32, tag="psqkv")
                                    for ks in range(KH):
                                        nc.tensor.matmul(
                                            ps, wk4[:, ks, pair * P:(pair + 1) * P],
                                            xnTc[t4][:, ks, :],
                                            start=(ks == 0), stop=(ks == KH - 1))
                                    nc.vector.tensor_scalar_add(
                                        out=KTp[:, t4 * SQ:(t4 + 1) * SQ], in0=ps,
                                        scalar1=bk_t[:, hp:hp + 1])

                                # both heads of the pair interleaved per key
                                # chunk (disjoint PE row groups 0/64).
                                cpss = [ps_ctx.tile([HD + 1, SQ], f32,
                                                    tag=f"ctxps{rh}",
                                                    name=f"ctxps{rh}")
                                        for rh in range(2)]
                                for kc in range(TT_S):
                                    pts = []
                                    for rh in range(2):
                                        rb = rh * HD
                                        sps = ps_s.tile([P, SQ], f32, tag="sps")
                                        nc.tensor.matmul(
                                            sps, KTp[rb:rb + HD, kc * P:(kc + 1) * P],
                                            QTp[pair][rb:rb + HD, :],
                                            start=True, stop=True)
                                        pt = pexp.tile([P, SQ], bf16, tag="pt")
                                        nc.scalar.activation(
                                            out=pt, in_=sps, func=AF.Exp,
                                            scale=float(1.0 / np.sqrt(HD)))
                                        pts.append(pt)
                                    for rh in range(2):
                                        hh = 2 * pair + rh
                                        nc.tensor.matmul(
                                            cpss[rh], va[:, kc, 4 * g4 + hh, :], pts[rh],
                                            start=(kc == 0), stop=(kc == TT_S - 1))
                                for rh in range(2):
                                    hh = 2 * pair + rh   # head within group
                                    h = 4 * g4 + hh      # global head
                                    cps = cpss[rh]
                                    # ctx[0:64,:] / l  (l = cps[64,:])
                                    rl = attn_sm.tile([1, SQ], f32r, tag="rl")
                                    with nc.allow_low_precision(reason="softmax denom bcast"):
                                        nc.vector.reciprocal(out=rl, in_=cps[HD:HD + 1, :])
                                    # PE broadcast of 1/l to 64 rows; the mul
                                    # reads it straight from PSUM
                                    bps = ps_b.tile([HD, SQ], f32, tag="bps")
                                    nc.tensor.matmul(bps, ones_t, rl,
                                                     start=True, stop=True)
                                    bsb = attn_sm.tile([HD, SQ], f32, tag="bsb")
                                    nc.vector.tensor_copy(out=bsb, in_=bps)
                                    slot = h // 2
                                    dst = ctxT[rh * HD:(rh + 1) * HD, slot, :]
                                    nc.vector.tensor_mul(dst, cps[0:HD, :], bsb)

            # xnT freed here.
            if True:
                # ---------------- Phase 3: output proj + residual ----------
                with (
                    tc.tile_pool(name="xrp", bufs=1) as xrp,
                    tc.tile_pool(name="ot", bufs=3) as otp,
                    tc.tile_pool(name="ps_o", bufs=3, space="PSUM") as ps_o,
                    tc.tile_pool(name="ps_t3", bufs=2, space="PSUM") as ps_t3,
                ):
                    xr = xrp.tile([P, TT_Q, H], f32)
                    for tt in range(TT_Q):
                        nc.sync.dma_start(out=xr[:, tt, :],
                                          in_=xp[tt * P:(tt + 1) * P, :])
                    oTs = {}

                    def emit_o_tail(os_):
                        # transposes + residual add + LN2 stats, one os_
                        # behind the matmuls so PE never waits on the ACT
                        # PSUM evacuation
                        oT = oTs.pop(os_)
                        pst = ps_t3.tile([P, TT_Q, P], f32r, tag="pst3")
                        for tt in range(TT_Q):
                            nc.tensor.transpose(pst[:, tt, :],
                                                oT[:, tt * P:(tt + 1) * P], ident_r)
                        nc.vector.tensor_add(
                            out_res[:, :, os_ * P:(os_ + 1) * P], pst,
                            xr[:, :, os_ * P:(os_ + 1) * P])
                        for tt in range(TT_Q):
                            nc.vector.bn_stats(
                                out=stats2[:, tt, os_, :],
                                in_=out_res[:, tt, os_ * P:(os_ + 1) * P])

                    for os_ in range(KH):
                        ps = ps_o.tile([P, SQ], f32, tag="pso")
                        for cs in range(KH):
                            nc.tensor.matmul(ps, wo_ts[os_][:, cs, :], ctxT[:, cs, :],
                                             start=(cs == 0), stop=(cs == KH - 1))
                        oT = otp.tile([P, SQ], f32r, tag="oT")
                        nc.scalar.activation(out=oT, in_=ps, func=AF.Identity,
                                             bias=bo2_t[:, os_:os_ + 1], scale=1.0)
                        oTs[os_] = oT
                        if os_ >= 1:
                            emit_o_tail(os_ - 1)
                    emit_o_tail(KH - 1)

                # release the wo/wqkv prefetch space (LIFO top); xn2T takes
                # that space for the FFN.
                prefetch_es.close()
                w1xp_es = ExitStack()
                w1xp = w1xp_es.enter_context(tc.tile_pool(name="w1xp", bufs=1))
                xn2T = w1xp.tile([P, KH, SQ], f32r)          # 16 KB/part

                # ---------------- Phase 4: LN2 -> xn2^T ----------------
                with (
                    tc.tile_pool(name="lnst2", bufs=4) as lnst2,
                    tc.tile_pool(name="ps_t4", bufs=2, space="PSUM") as ps_t4,
                ):
                    # batched scalar chain for all 4 tiles (one hop each)
                    mv4 = lnst2.tile([P, TT_Q, 2], f32)
                    for i in range(TT_Q):
                        nc.vector.bn_aggr(out=mv4[:, i, :], in_=stats2[:, i, :, :])
                    rstd4 = lnst2.tile([P, TT_Q], f32)
                    nc.scalar.activation(out=rstd4, in_=mv4[:, :, 1], func=AF.Sqrt,
                                         bias=eps_t, scale=1.0)
                    nc.vector.reciprocal(out=rstd4, in_=rstd4)
                    negmr4 = lnst2.tile([P, TT_Q], f32)
                    nc.vector.scalar_tensor_tensor(
                        out=negmr4, in0=mv4[:, :, 0], scalar=negone, in1=rstd4,
                        op0=ALU.mult, op1=ALU.mult)
                    for i in range(TT_Q):
                        xn2 = lnst2.tile([P, H], f32r, tag="xn2")
                        nc.scalar.activation(out=xn2, in_=out_res[:, i, :], func=AF.Identity,
                                             bias=negmr4[:, i:i + 1], scale=rstd4[:, i:i + 1])
                        for j2 in range(2):
                            ps = ps_t4.tile([P, 4, P], f32r, tag="pst4")
                            for k in range(4):
                                j = 4 * j2 + k
                                nc.tensor.transpose(
                                    ps[:, k, :], xn2[:, j * P:(j + 1) * P], ident_r)
                            dst = xn2T[:, 4 * j2:4 * j2 + 4, i * P:(i + 1) * P]
                            nc.vector.tensor_copy(out=dst, in_=ps)

                # ---------------- Phase 5: FFN ----------------
                with tc.tile_pool(name="hTp", bufs=1) as hTp:
                    hT = hTp.tile([P, FF // P, SQ], f32r)   # 64 KB/part
                    with (
                        tc.tile_pool(name="w2p", bufs=6) as w2p,
                        tc.tile_pool(name="ps_f", bufs=3, space="PSUM") as ps_f,
                    ):
                        for ft in range(FF // P):
                            if ft in w1_ts:
                                w1_t = w1_ts.pop(ft)
                            else:
                                w1_t = w1p.tile([P, KH, P], f32r, tag="w1_t",
                                                name="w1_t")
                                nc.sync.dma_start(
                                    out=w1_t,
                                    in_=W1.rearrange("(a p) c -> p a c", p=P)[:, :, ft * P:(ft + 1) * P].bitcast(f32r))
                            ps = ps_f.tile([P, SQ], f32, tag="psf")
                            for ks in range(KH):
                                nc.tensor.matmul(ps, w1_t[:, ks, :], xn2T[:, ks, :],
                                                 start=(ks == 0), stop=(ks == KH - 1))
                            nc.scalar.activation(out=hT[:, ft, :], in_=ps, func=AF.Relu,
                                                 bias=bf1_t[:, ft:ft + 1], scale=1.0)

                        with (
                            tc.tile_pool(name="o2", bufs=3) as o2p,
                            tc.tile_pool(name="yout", bufs=2) as youtp,
                            tc.tile_pool(name="ps_o2", bufs=2, space="PSUM") as ps_o2,
                            tc.tile_pool(name="ps_t5", bufs=2, space="PSUM") as ps_t5,
                        ):
                            o2Ts = {}

                            def emit_y_tail(os_):
                                o2T = o2Ts.pop(os_)
                                pst = ps_t5.tile([P, TT_Q, P], f32r, tag="pst5")
                                for tt in range(TT_Q):
                                    nc.tensor.transpose(pst[:, tt, :],
                                                        o2T[:, tt * P:(tt + 1) * P], ident_r)
                                yo = youtp.tile([P, TT_Q, P], f32, tag="yo")
                                nc.vector.tensor_add(
                                    yo, pst, out_res[:, :, os_ * P:(os_ + 1) * P])
                                nc.sync.dma_start(
                                    out=y.rearrange("(t p) c -> p t c", p=P)[:, :, os_ * P:(os_ + 1) * P],
                                    in_=yo)

                            for os_ in range(KH):
                                ps = ps_o2.tile([P, SQ], f32, tag="pso2")
                                for q4 in range(4):
                                    w2_t = w2p.tile([P, 8, P], f32r, tag="w2_t")
                                    dma_eng = nc.gpsimd if q4 % 2 == 0 else nc.sync
                                    dma_eng.dma_start(
                                        out=w2_t,
                                        in_=W2.rearrange("(a p) c -> p a c", p=P)[:, q4 * 8:(q4 + 1) * 8, os_ * P:(os_ + 1) * P].bitcast(f32r))
                                    for f8 in range(8):
                                        ft = q4 * 8 + f8
                                        nc.tensor.matmul(ps, w2_t[:, f8, :], hT[:, ft, :],
                                                         start=(ft == 0), stop=(ft == FF // P - 1))
                                o2T = o2p.tile([P, SQ], f32r, tag="o2T")
                                nc.scalar.activation(out=o2T, in_=ps, func=AF.Identity,
                                                     bias=bf2_t[:, os_:os_ + 1], scale=1.0)
                                o2Ts[os_] = o2T
                                if os_ >= 1:
                                    emit_y_tail(os_ - 1)
                            emit_y_tail(KH - 1)

                # LIFO pool teardown for the manually-entered pools
                w1xp_es.close()
                prefetch_es2 = None  # (placeholder, nothing else open here)
                w1p_es.close()
                persistB_es.close()

    nc.finalize()
    return nc


_NC_CACHE = {}


def kernel(**inputs):
    import concourse.bass_utils as bass_utils
    import ml_dtypes

    x = np.ascontiguousarray(np.asarray(inputs["x"], dtype=np.float32))
    f64 = np.float64
    g1 = np.asarray(inputs["g1"], f64)
    b1 = np.asarray(inputs["b1"], f64)
    g2 = np.asarray(inputs["g2"], f64)
    b2 = np.asarray(inputs["b2"], f64)
    Wq = np.asarray(inputs["Wq"], f64)
    Wk = np.asarray(inputs["Wk"], f64)
    Wv = np.asarray(inputs["Wv"], f64)
    Wo = np.asarray(inputs["Wo"], f64)
    W1 = np.asarray(inputs["W1"], f64)

    bf16 = ml_dtypes.bfloat16
    # exact affine folds (see module docstring)
    Wq_f = (g1[:, None] * Wq).astype(np.float32).astype(bf16)
    Wk_f = (g1[:, None] * Wk).astype(np.float32).astype(bf16)
    Wv_f = (g1[:, None] * Wv).astype(np.float32).astype(bf16)
    W1_f = (g2[:, None] * W1).astype(np.float32)
    bq_f = (np.asarray(inputs["bq"], f64) + b1 @ Wq).astype(np.float32)
    bk_f = (np.asarray(inputs["bk"], f64) + b1 @ Wk).astype(np.float32)
    bv_f = np.asarray(inputs["bv"], f64) + b1 @ Wv
    bo2 = (np.asarray(inputs["bo"], f64) + bv_f @ Wo).astype(np.float32)
    bf1_f = (np.asarray(inputs["bf1"], f64) + b2 @ W1).astype(np.float32)

    weights = {
        "Wq": np.ascontiguousarray(Wq_f), "Wk": np.ascontiguousarray(Wk_f),
        "Wv": np.ascontiguousarray(Wv_f),
        "Wo": np.ascontiguousarray(np.asarray(inputs["Wo"], np.float32)),
        "W1": np.ascontiguousarray(W1_f),
        "W2": np.ascontiguousarray(np.asarray(inputs["W2"], np.float32)),
        "bq": bq_f, "bk": bk_f, "bo2": bo2, "bf1": bf1_f,
        "bf2": np.ascontiguousarray(np.asarray(inputs["bf2"], np.float32)),
    }

    if "nc" not in _NC_CACHE:
        _NC_CACHE["nc"] = _build_nc()
    nc = _NC_CACHE["nc"]

    in_maps = []
    for c in range(N_CORES):
        b, qc = c // 4, c % 4
        xb = np.roll(x[b], -qc * SQ, axis=0)
        m = {"xp": np.ascontiguousarray(xb)}
        m.update(weights)
        in_maps.append(m)

    res = bass_utils.run_bass_kernel_spmd(nc, in_maps, core_ids=list(range(N_CORES)))
    out = np.empty((B, S, H), dtype=np.float32)
    for c in range(N_CORES):
        b, qc = c // 4, c % 4
        out[b, qc * SQ:(qc + 1) * SQ, :] = res.results[c]["y"]
    return out


# revision 14
# speedup vs baseline: 1.0010x; 1.0010x over previous
"""Trainium2 Bass kernel for a dense transformer AttentionBlock (optimized v2).

Problem: x[2,2048,1024] fp32 -> LN1 -> MHA(16 heads, hd=64) + residual
         -> LN2 -> FFN(4096, relu) + residual.

Sharding: 8-way data parallel. Core c handles batch b=c//4 and query chunk
qc=c%4 (512 tokens). K/V are computed for the full 2048-token sequence on
each core (redundant within a batch group, but no collectives needed).
The host rotates each core's token axis so its query chunk is rows 0:512 —
softmax over keys is permutation invariant so key order does not matter.

Host-side exact weight algebra (one-time O(H^2) prep, keeps device math
identical): LN affines are folded into the projection weights
(Wq' = g1*Wq rows, bq' = bq + b1@Wq, same for K/V; W1' = g2*W1,
bf1' = bf1 + b2@W1) and the V bias is folded through the output projection
(bo2 = bo + bv'@Wo, exact because softmax rows sum to 1). Wq/Wk/Wv ship
pre-cast to bf16 (the attention matmul dtype) to halve their DMA bytes.

Precision: FFN / output-proj matmuls in float32r (full PE rate at N>=512);
attention datapath bf16 with fp32 PSUM. Softmax uses exp without max
subtraction; denominator accumulated via a ones-column in the V operand and
divided out per head (PE broadcast of the reciprocal row).

Scheduling notes (tuned against the TimelineSim cost model): all DMA
transfers serialize on one ~360 GB/s pipe, so x tiles are dispatched ahead
of Wv; Wo / x-residual / first W1 tiles live in pools opened before phase 1
so their SP-queue DMAs (emitted after phase 1) stream during attention; LN
runs two tiles ahead of the transpose+V work so PE never idles (idle gaps
cost 2-3.7x p-state matmul penalties for the next ~3us).
"""

import numpy as np
from contextlib import ExitStack

B, S, H = 2, 2048, 1024
NH, HD = 16, 64
FF = 4 * H
EPS = 1e-5
P = 128
SQ = 512          # query-chunk tokens per core
N_CORES = 8
TT_Q = SQ // P    # 4 token tiles in the query chunk
TT_S = S // P     # 16 token tiles in the full sequence
KH = H // P       # 8 k-tiles over hidden dim


def _build_nc():
    import concourse.bass as bass
    import concourse.mybir as mybir
    import concourse.tile as tile
    from concourse import bacc
    from concourse.masks import make_identity

    dt = mybir.dt
    f32 = dt.float32
    f32r = dt.float32r
    bf16 = dt.bfloat16
    AF = mybir.ActivationFunctionType
    ALU = mybir.AluOpType

    nc = bacc.Bacc(None, target_bir_lowering=False)

    xp = nc.dram_tensor("xp", [S, H], f32, kind="ExternalInput")
    Wq = nc.dram_tensor("Wq", [H, H], bf16, kind="ExternalInput")
    Wk = nc.dram_tensor("Wk", [H, H], bf16, kind="ExternalInput")
    Wv = nc.dram_tensor("Wv", [H, H], bf16, kind="ExternalInput")
    Wo = nc.dram_tensor("Wo", [H, H], f32, kind="ExternalInput")
    W1 = nc.dram_tensor("W1", [H, FF], f32, kind="ExternalInput")
    W2 = nc.dram_tensor("W2", [FF, H], f32, kind="ExternalInput")
    bq = nc.dram_tensor("bq", [H], f32, kind="ExternalInput")
    bk = nc.dram_tensor("bk", [H], f32, kind="ExternalInput")
    bo2 = nc.dram_tensor("bo2", [H], f32, kind="ExternalInput")
    bf1 = nc.dram_tensor("bf1", [FF], f32, kind="ExternalInput")
    bf2 = nc.dram_tensor("bf2", [H], f32, kind="ExternalInput")
    y = nc.dram_tensor("y", [SQ, H], f32, kind="ExternalOutput")

    def col_view(dram_vec, n):
        # DRAM [n*P] viewed as [P, n]: element [p, j] = vec[j*P + p]
        return dram_vec.rearrange("(a p) -> p a", p=P)

    def layernorm_tile(pool, src_ap, tag, xn_dt=f32):
        """token-major LN of one [P, H] tile -> (x - m) * rstd. The per
        feature g/b affine is folded into the weights on the host."""
        stats = pool.tile([P, 2, 6], f32, tag=tag + "st")
        nc.vector.bn_stats(out=stats[:, 0, :], in_=src_ap[:, 0:512])
        nc.vector.bn_stats(out=stats[:, 1, :], in_=src_ap[:, 512:1024])
        mv = pool.tile([P, 2], f32, tag=tag + "mv")
        nc.vector.bn_aggr(out=mv, in_=stats)
        rstd = pool.tile([P, 1], f32, tag=tag + "rs")
        nc.scalar.activation(out=rstd, in_=mv[:, 1:2], func=AF.Sqrt,
                             bias=eps_t, scale=1.0)
        nc.vector.reciprocal(out=rstd, in_=rstd)
        negmr = pool.tile([P, 1], f32, tag=tag + "nm")
        nc.vector.scalar_tensor_tensor(
            out=negmr, in0=mv[:, 0:1], scalar=rstd, in1=negone,
            op0=ALU.mult, op1=ALU.mult)
        xn = pool.tile([P, H], xn_dt, tag=tag + "xn")
        nc.scalar.activation(out=xn, in_=src_ap, func=AF.Identity,
                             bias=negmr, scale=rstd)
        return xn

    with tile.TileContext(nc) as tc:
        with (
            tc.tile_pool(name="consts", bufs=1) as consts,
            tc.tile_pool(name="persistA", bufs=1) as persistA,
        ):
            ctxT = persistA.tile([P, KH, SQ], f32r)      # 16 KB/part

            ident_b = consts.tile([P, P], bf16)
            make_identity(nc, ident_b)
            eps_t = consts.tile([P, 1], f32)
            nc.vector.memset(eps_t, EPS)
            negone = consts.tile([P, 1], f32)
            nc.vector.memset(negone, -1.0)
            ident_f = consts.tile([P, P], f32)
            make_identity(nc, ident_f)
            ident_r = consts.tile([P, P], f32r)
            nc.scalar.copy(out=ident_r, in_=ident_f)
            # f32r constant 1.0 (memset can't emit f32r; ACT Copy(0*x+1) can;
            # the input is ignored at scale=0)
            ones_t = consts.tile([1, HD], f32r)
            nc.scalar.activation(out=ones_t, in_=ident_f[0:1, 0:HD], func=AF.Copy,
                                 bias=1.0, scale=0.0)

            # persistB (phase-3+ accumulators) and the prefetch pools open
            # BEFORE phase 1 so they get SBUF space disjoint from the
            # phase-1/2 pools: their SP-queue DMAs (emitted after the
            # phase-1 loop) then stream during attention with no space-reuse
            # dependencies. prefetch_es closes right after phase 3 (LIFO) to
            # make room for the FFN pools.
            persistB_es = ExitStack()
            persistB = persistB_es.enter_context(tc.tile_pool(name="persistB", bufs=1))
            out_res = persistB.tile([P, TT_Q, H], f32)   # 16 KB/part
            stats2 = persistB.tile([P, TT_Q, KH, 6], f32)

            w1p_es = ExitStack()
            w1p = w1p_es.enter_context(tc.tile_pool(name="w1p", bufs=4))

            prefetch_es = ExitStack()
            wop = prefetch_es.enter_context(tc.tile_pool(name="wo", bufs=1))
            wqkv = prefetch_es.enter_context(tc.tile_pool(name="wqkv", bufs=2))
            qtp = prefetch_es.enter_context(tc.tile_pool(name="qt", bufs=2))

            with tc.tile_pool(name="xnTp", bufs=1) as xnTp:
                xnTc = [xnTp.tile([P, KH, SQ], bf16, tag=f"xnt{c}", name=f"xnt{c}")
                        for c in range(4)]               # 4 x 8 KB/part

                with tc.tile_pool(name="vap", bufs=1) as vap:
                    # token-major V (+ ones column for softmax denominator)
                    va = vap.tile([P, TT_S, NH, HD + 1], bf16)
                    nc.vector.memset(va[:, :, :, HD:HD + 1], 1.0)

                    # ------- Phase 1: LN1 + transpose + V projection -------
                    with (
                        tc.tile_pool(name="ln", bufs=4) as ln,
                        tc.tile_pool(name="lnst", bufs=4) as lnst,
                        tc.tile_pool(name="wvp", bufs=1) as wvp,
                        tc.tile_pool(name="ps_t", bufs=2, space="PSUM") as ps_t,
                        tc.tile_pool(name="ps_v", bufs=2, space="PSUM") as ps_v,
                    ):
                        # LN is software-pipelined two tiles ahead of the
                        # transpose/V work so PE never waits on the LN chain.
                        xns = {}

                        def emit_ln(i):
                            xt = ln.tile([P, H], f32, tag="xt", name="xt")
                            nc.sync.dma_start(out=xt, in_=xp[i * P:(i + 1) * P, :])
                            xns[i] = layernorm_tile(lnst, xt, "l1", xn_dt=bf16)

                        emit_ln(0)
                        emit_ln(1)
                        emit_ln(2)

                        # Wv quarters queue behind the first x tiles on the
                        # (serial) DMA pipe.
                        wv = wvp.tile([P, KH, H], bf16)
                        wv_view = Wv.rearrange("(a p) c -> p a c", p=P)
                        nc.scalar.dma_start(out=wv[:, :, 0:256], in_=wv_view[:, :, 0:256])
                        nc.gpsimd.dma_start(out=wv[:, :, 256:512], in_=wv_view[:, :, 256:512])
                        nc.scalar.dma_start(out=wv[:, :, 512:768], in_=wv_view[:, :, 512:768])
                        nc.gpsimd.dma_start(out=wv[:, :, 768:1024], in_=wv_view[:, :, 768:1024])

                        bq_t = consts.tile([P, KH], f32)
                        bk_t = consts.tile([P, KH], f32)
                        bo2_t = consts.tile([P, KH], f32)
                        bf2_t = consts.tile([P, KH], f32)
                        bf1_t = consts.tile([P, FF // P], f32)
                        nc.gpsimd.dma_start(out=bq_t, in_=col_view(bq, KH))
                        nc.gpsimd.dma_start(out=bk_t, in_=col_view(bk, KH))
                        nc.gpsimd.dma_start(out=bo2_t, in_=col_view(bo2, KH))
                        nc.gpsimd.dma_start(out=bf2_t, in_=col_view(bf2, KH))
                        nc.gpsimd.dma_start(out=bf1_t, in_=col_view(bf1, FF // P))

                        def emit_v(i):
                            # V projection for tile i (runs one tile behind
                            # the transposes so PE never waits on the
                            # PSUM->SBUF evacuation of xnT)
                            for fg in range(2):
                                psw = ps_v.tile([P, SQ], f32, tag="psv",
                                                name="psw")
                                for ks in range(KH):
                                    nc.tensor.matmul(
                                        psw, xnTc[i // 4][:, ks, (i % 4) * P:(i % 4 + 1) * P],
                                        wv[:, ks, fg * SQ:(fg + 1) * SQ],
                                        start=(ks == 0), stop=(ks == KH - 1))
                                dstv = va[:, i, 8 * fg:8 * fg + 8, 0:HD]
                                src = psw.rearrange("p (h d) -> p h d", d=HD)
                                if fg == 0:
                                    nc.vector.tensor_copy(out=dstv, in_=src)
                                else:
                                    nc.scalar.copy(out=dstv, in_=src)

                        def emit_q0():
                            # Q projection for head-group 0 (only needs
                            # xnTc[0], ready after tile 3) — fills the PE
                            # starvation window while LN1 still paces tiles.
                            wk4 = wqkv.tile([P, KH, 256], bf16, tag="wk4",
                                            name="wk4")
                            wq4 = wqkv.tile([P, KH, 256], bf16, tag="wq4",
                                            name="wq4")
                            nc.gpsimd.dma_start(
                                out=wk4,
                                in_=Wk.rearrange("(a p) c -> p a c", p=P)[:, :, 0:256])
                            nc.gpsimd.dma_start(
                                out=wq4,
                                in_=Wq.rearrange("(a p) c -> p a c", p=P)[:, :, 0:256])
                            QTp = []
                            for pair in range(2):
                                ps = ps_v.tile([P, SQ], f32, tag="psv",
                                               name="psq0")
                                for ks in range(KH):
                                    nc.tensor.matmul(
                                        ps, wq4[:, ks, pair * P:(pair + 1) * P],
                                        xnTc[0][:, ks, :],
                                        start=(ks == 0), stop=(ks == KH - 1))
                                qt0 = qtp.tile([P, SQ], bf16, tag=f"qtp{pair}",
                                               name=f"qtp{pair}")
                                nc.vector.tensor_scalar_add(
                                    out=qt0, in0=ps, scalar1=bq_t[:, pair:pair + 1])
                                QTp.append(qt0)
                            return wk4, wq4, QTp


                        g0_hoist = {}
                        for i in range(TT_S):
                            if i + 3 < TT_S:
                                emit_ln(i + 3)
                            if i == 4:
                                g0_hoist["v"] = emit_q0()
                            xn = xns.pop(i)
                            for j2 in range(2):
                                ps = ps_t.tile([P, 4, P], bf16, tag="pst")
                                for k in range(4):
                                    j = 4 * j2 + k
                                    nc.tensor.transpose(
                                        ps[:, k, :], xn[:, j * P:(j + 1) * P], ident_b)
                                dst = xnTc[i // 4][:, 4 * j2:4 * j2 + 4,
                                                  (i % 4) * P:(i % 4 + 1) * P]
                                if j2 == 0:
                                    nc.vector.tensor_copy(out=dst, in_=ps)
                                else:
                                    nc.scalar.copy(out=dst, in_=ps)
                            if i >= 1:
                                emit_v(i - 1)
                        emit_v(TT_S - 1)

                    # Prefetch DMAs for phase 3+ (SP queue drains these
                    # during attention; pools were opened before phase 1).
                    wo_ts = []
                    for os_ in range(KH):
                        wo_t = wop.tile([P, KH, P], f32r, tag=f"wo{os_}",
                                        name=f"wo{os_}")
                        nc.sync.dma_start(
                            out=wo_t,
                            in_=Wo.rearrange("(a p) c -> p a c", p=P)[:, :, os_ * P:(os_ + 1) * P].bitcast(f32r))
                        wo_ts.append(wo_t)
                    w1_ts = {}
                    for ft in range(4):
                        w1_t = w1p.tile([P, KH, P], f32r, tag="w1_t", name="w1_t")
                        nc.sync.dma_start(
                            out=w1_t,
                            in_=W1.rearrange("(a p) c -> p a c", p=P)[:, :, ft * P:(ft + 1) * P].bitcast(f32r))
                        w1_ts[ft] = w1_t

                    # ---------------- Phase 2: K/Q + attention ----------------
                    with ExitStack() as es2:
                        ktp = es2.enter_context(tc.tile_pool(name="kt", bufs=2))
                        pexp = es2.enter_context(tc.tile_pool(name="pexp", bufs=3))
                        attn_sm = es2.enter_context(tc.tile_pool(name="attn_sm", bufs=2))
                        ps_qkv = es2.enter_context(tc.tile_pool(name="ps_qkv", bufs=2, space="PSUM"))
                        ps_s = es2.enter_context(tc.tile_pool(name="ps_s", bufs=3, space="PSUM"))
                        ps_ctx = es2.enter_context(tc.tile_pool(name="ps_ctx", bufs=1, space="PSUM"))
                        ps_b = es2.enter_context(tc.tile_pool(name="ps_b", bufs=1, space="PSUM"))
                        for g4 in range(4):
                            c0 = g4 * 4 * HD  # first feature column of this group
                            if g4 == 0:
                                wk4, wq4, QTp = g0_hoist.pop("v")
                            else:
                                wk4 = wqkv.tile([P, KH, 256], bf16, tag="wk4")
                                wq4 = wqkv.tile([P, KH, 256], bf16, tag="wq4")
                                nc.gpsimd.dma_start(
                                    out=wk4,
                                    in_=Wk.rearrange("(a p) c -> p a c", p=P)[:, :, c0:c0 + 256])
                                nc.gpsimd.dma_start(
                                    out=wq4,
                                    in_=Wq.rearrange("(a p) c -> p a c", p=P)[:, :, c0:c0 + 256])

                                # Q^T pair-stacked: [128(2 heads), SQ] per pair
                                QTp = [qtp.tile([P, SQ], bf16, tag=f"qtp{pair}",
                                                name=f"qtp{pair}") for pair in range(2)]
                                for pair in range(2):
                                    hp = 2 * g4 + pair
                                    ps = ps_qkv.tile([P, SQ], f32, tag="psqkv")
                                    for ks in range(KH):
                                        nc.tensor.matmul(
                                            ps, wq4[:, ks, pair * P:(pair + 1) * P],
                                            xnTc[0][:, ks, :],
                                            start=(ks == 0), stop=(ks == KH - 1))
                                    nc.vector.tensor_scalar_add(
                                        out=QTp[pair], in0=ps,
                                        scalar1=bq_t[:, hp:hp + 1])

                            for pair in range(2):
                                hp = 2 * g4 + pair
                                # K^T pair-stacked: [128(2 heads), S]
                                KTp = ktp.tile([P, S], bf16, tag="ktp")
                                for t4 in range(4):
                                    ps = ps_qkv.tile([P, SQ], f32, tag="psqkv")
                                    for ks in range(KH):
                                        nc.tensor.matmul(
                                            ps, wk4[:, ks, pair * P:(pair + 1) * P],
                                            xnTc[t4][:, ks, :],
                                            start=(ks == 0), stop=(ks == KH - 1))
                                    nc.vector.tensor_scalar_add(
                                        out=KTp[:, t4 * SQ:(t4 + 1) * SQ], in0=ps,
                                        scalar1=bk_t[:, hp:hp + 1])

                                # both heads of the pair interleaved per key
                                # chunk (disjoint PE row groups 0/64).
                                cpss = [ps_ctx.tile([HD + 1, SQ], f32,
                                                    tag=f"ctxps{rh}",
                                                    name=f"ctxps{rh}")
                                        for rh in range(2)]
                                for kc in range(TT_S):
                                    pts = []
                                    for rh in range(2):
                                        rb = rh * HD
                                        sps = ps_s.tile([P, SQ], f32, tag="sps")
                                        nc.tensor.matmul(
                                            sps, KTp[rb:rb + HD, kc * P:(kc + 1) * P],
                                            QTp[pair][rb:rb + HD, :],
                                            start=True, stop=True)
                                        pt = pexp.tile([P, SQ], bf16, tag="pt")
                                        nc.scalar.activation(
                                            out=pt, in_=sps, func=AF.Exp,
                                            scale=float(1.0 / np.sqrt(HD)))
                                        pts.append(pt)
                                    for rh in range(2):
                                        hh = 2 * pair + rh
                                        nc.tensor.matmul(
                                            cpss[rh], va[:, kc, 4 * g4 + hh, :], pts[rh],
                                            start=(kc == 0), stop=(kc == TT_S - 1))
                                for rh in range(2):
                                    hh = 2 * pair + rh   # head within group
                                    h = 4 * g4 + hh      # global head
                                    cps = cpss[rh]
                                    # ctx[0:64,:] / l  (l = cps[64,:])
                                    rl = attn_sm.tile([1, SQ], f32r, tag="rl")
                                    with nc.allow_low_precision(reason="softmax denom bcast"):
                                        nc.vector.reciprocal(out=rl, in_=cps[HD:HD + 1, :])
                                    # PE broadcast of 1/l to 64 rows; the mul
                                    # reads it straight from PSUM
                                    bps = ps_b.tile([HD, SQ], f32, tag="bps")
                                    nc.tensor.matmul(bps, ones_t, rl,
                                                     start=True, stop=True)
                                    bsb = attn_sm.tile([HD, SQ], f32, tag="bsb")
                                    nc.vector.tensor_copy(out=bsb, in_=bps)
                                    slot = h // 2
                                    dst = ctxT[rh * HD:(rh + 1) * HD, slot, :]
                                    nc.vector.tensor_mul(dst, cps[0:HD, :], bsb)

            # xnT freed here.
            if True:
                # ---------------- Phase 3: output proj + residual ----------
                with (
                    tc.tile_pool(name="xrp", bufs=1) as xrp,
                    tc.tile_pool(name="ot", bufs=3) as otp,
                    tc.tile_pool(name="ps_o", bufs=3, space="PSUM") as ps_o,
                    tc.tile_pool(name="ps_t3", bufs=2, space="PSUM") as ps_t3,
                ):
                    xr = xrp.tile([P, TT_Q, H], f32)
                    for tt in range(TT_Q):
                        nc.sync.dma_start(out=xr[:, tt, :],
                                          in_=xp[tt * P:(tt + 1) * P, :])
                    oTs = {}

                    def emit_o_tail(os_):
                        # transposes + residual add + LN2 stats, one os_
                        # behind the matmuls so PE never waits on the ACT
                        # PSUM evacuation
                        oT = oTs.pop(os_)
                        pst = ps_t3.tile([P, TT_Q, P], f32r, tag="pst3")
                        for tt in range(TT_Q):
                            nc.tensor.transpose(pst[:, tt, :],
                                                oT[:, tt * P:(tt + 1) * P], ident_r)
                        nc.vector.tensor_add(
                            out_res[:, :, os_ * P:(os_ + 1) * P], pst,
                            xr[:, :, os_ * P:(os_ + 1) * P])
                        for tt in range(TT_Q):
                            nc.vector.bn_stats(
                                out=stats2[:, tt, os_, :],
                                in_=out_res[:, tt, os_ * P:(os_ + 1) * P])

                    for os_ in range(KH):
                        ps = ps_o.tile([P, SQ], f32, tag="pso")
                        for cs in range(KH):
                            nc.tensor.matmul(ps, wo_ts[os_][:, cs, :], ctxT[:, cs, :],
                                             start=(cs == 0), stop=(cs == KH - 1))
                        oT = otp.tile([P, SQ], f32r, tag="oT")
                        nc.scalar.activation(out=oT, in_=ps, func=AF.Identity,
                                             bias=bo2_t[:, os_:os_ + 1], scale=1.0)
                        oTs[os_] = oT
                        if os_ >= 1:
                            emit_o_tail(os_ - 1)
                    emit_o_tail(KH - 1)

                # release the wo/wqkv prefetch space (LIFO top); xn2T takes
                # that space for the FFN.
                prefetch_es.close()
                w1xp_es = ExitStack()
                w1xp = w1xp_es.enter_context(tc.tile_pool(name="w1xp", bufs=1))
                xn2T = w1xp.tile([P, KH, SQ], f32r)          # 16 KB/part

                # ---------------- Phase 4: LN2 -> xn2^T ----------------
                with (
                    tc.tile_pool(name="lnst2", bufs=4) as lnst2,
                    tc.tile_pool(name="ps_t4", bufs=2, space="PSUM") as ps_t4,
                ):
                    # batched scalar chain for all 4 tiles (one hop each)
                    mv4 = lnst2.tile([P, TT_Q, 2], f32)
                    for i in range(TT_Q):
                        nc.vector.bn_aggr(out=mv4[:, i, :], in_=stats2[:, i, :, :])
                    rstd4 = lnst2.tile([P, TT_Q], f32)
                    nc.scalar.activation(out=rstd4, in_=mv4[:, :, 1], func=AF.Sqrt,
                                         bias=eps_t, scale=1.0)
                    nc.vector.reciprocal(out=rstd4, in_=rstd4)
                    negmr4 = lnst2.tile([P, TT_Q], f32)
                    nc.vector.scalar_tensor_tensor(
                        out=negmr4, in0=mv4[:, :, 0], scalar=negone, in1=rstd4,
                        op0=ALU.mult, op1=ALU.mult)
                    for i in range(TT_Q):
                        xn2 = lnst2.tile([P, H], f32r, tag="xn2")
                        nc.scalar.activation(out=xn2, in_=out_res[:, i, :], func=AF.Identity,
                                             bias=negmr4[:, i:i + 1], scale=rstd4[:, i:i + 1])
                        for j2 in range(2):
                            ps = ps_t4.tile([P, 4, P], f32r, tag="pst4")
                            for k in range(4):
                                j = 4 * j2 + k
                                nc.tensor.transpose(
                                    ps[:, k, :], xn2[:, j * P:(j + 1) * P], ident_r)
                            dst = xn2T[:, 4 * j2:4 * j2 + 4, i * P:(i + 1) * P]
                            nc.vector.tensor_copy(out=dst, in_=ps)

                # ---------------- Phase 5: FFN ----------------
                with tc.tile_pool(name="hTp", bufs=1) as hTp:
                    hT = hTp.tile([P, FF // P, SQ], f32r)   # 64 KB/part
                    with (
                        tc.tile_pool(name="w2p", bufs=6) as w2p,
                        tc.tile_pool(name="ps_f", bufs=3, space="PSUM") as ps_f,
                    ):
                        for ft in range(FF // P):
                            if ft in w1_ts:
                                w1_t = w1_ts.pop(ft)
                            else:
                                w1_t = w1p.tile([P, KH, P], f32r, tag="w1_t",
                                                name="w1_t")
                                nc.sync.dma_start(
                                    out=w1_t,
                                    in_=W1.rearrange("(a p) c -> p a c", p=P)[:, :, ft * P:(ft + 1) * P].bitcast(f32r))
                            ps = ps_f.tile([P, SQ], f32, tag="psf")
                            for ks in range(KH):
                                nc.tensor.matmul(ps, w1_t[:, ks, :], xn2T[:, ks, :],
                                                 start=(ks == 0), stop=(ks == KH - 1))
                            nc.scalar.activation(out=hT[:, ft, :], in_=ps, func=AF.Relu,
                                                 bias=bf1_t[:, ft:ft + 1], scale=1.0)

                        with (
                            tc.tile_pool(name="o2", bufs=3) as o2p,
                            tc.tile_pool(name="yout", bufs=2) as youtp,
                            tc.tile_pool(name="ps_o2", bufs=2, space="PSUM") as ps_o2,
                            tc.tile_pool(name="ps_t5", bufs=2, space="PSUM") as ps_t5,
                        ):
                            o2Ts = {}

                            def emit_y_tail(os_):
                                o2T = o2Ts.pop(os_)
                                pst = ps_t5.tile([P, TT_Q, P], f32r, tag="pst5")
                                for tt in range(TT_Q):
                                    nc.tensor.transpose(pst[:, tt, :],
                                                        o2T[:, tt * P:(tt + 1) * P], ident_r)
                                yo = youtp.tile([P, TT_Q, P], f32, tag="yo")
                                nc.vector.tensor_add(
                                    yo, pst, out_res[:, :, os_ * P:(os_ + 1) * P])
                                nc.sync.dma_start(
                                    out=y.rearrange("(t p) c -> p t c", p=P)[:, :, os_ * P:(os_ + 1) * P],
                                    in_=yo)

                            for os_ in range(KH):
                                ps = ps_o2.tile([P, SQ], f32, tag="pso2")
                                for q4 in range(4):
                                    w2_t = w2p.tile([P, 8, P], f32r, tag="w2_t")
                                    dma_eng = nc.gpsimd if q4 % 2 == 0 else nc.sync
                                    dma_eng.dma_start(
                                        out=w2_t,
                                        in_=W2.rearrange("(a p) c -> p a c", p=P)[:, q4 * 8:(q4 + 1) * 8, os_ * P:(os_ + 1) * P].bitcast(f32r))
                                    for f8 in range(8):
                                        ft = q4 * 8 + f8
                                        nc.tensor.matmul(ps, w2_t[:, f8, :], hT[:, ft, :],
                                                         start=(ft == 0), stop=(ft == FF // P - 1))
                                o2T = o2p.tile([P, SQ], f32r, tag="o2T")
                                nc.scalar.activation(out=o2T, in_=ps, func=AF.Identity,
                                                     bias=bf2_t[:, os_:os_ + 1], scale=1.0)
                                o2Ts[os_] = o2T
                                if os_ >= 1:
                                    emit_y_tail(os_ - 1)
                            emit_y_tail(KH - 1)

                # LIFO pool teardown for the manually-entered pools
                w1xp_es.close()
                prefetch_es2 = None  # (placeholder, nothing else open here)
                w1p_es.close()
                persistB_es.close()

    nc.finalize()
    return nc


_NC_CACHE = {}


def kernel(**inputs):
    import concourse.bass_utils as bass_utils
    import ml_dtypes

    x = np.ascontiguousarray(np.asarray(inputs["x"], dtype=np.float32))
    f64 = np.float64
    g1 = np.asarray(inputs["g1"], f64)
    b1 = np.asarray(inputs["b1"], f64)
    g2 = np.asarray(inputs["g2"], f64)
    b2 = np.asarray(inputs["b2"], f64)
    Wq = np.asarray(inputs["Wq"], f64)
    Wk = np.asarray(inputs["Wk"], f64)
    Wv = np.asarray(inputs["Wv"], f64)
    Wo = np.asarray(inputs["Wo"], f64)
    W1 = np.asarray(inputs["W1"], f64)

    bf16 = ml_dtypes.bfloat16
    # exact affine folds (see module docstring)
    Wq_f = (g1[:, None] * Wq).astype(np.float32).astype(bf16)
    Wk_f = (g1[:, None] * Wk).astype(np.float32).astype(bf16)
    Wv_f = (g1[:, None] * Wv).astype(np.float32).astype(bf16)
    W1_f = (g2[:, None] * W1).astype(np.float32)
    bq_f = (np.asarray(inputs["bq"], f64) + b1 @ Wq).astype(np.float32)
    bk_f = (np.asarray(inputs["bk"], f64) + b1 @ Wk).astype(np.float32)
    bv_f = np.asarray(inputs["bv"], f64) + b1 @ Wv
    bo2 = (np.asarray(inputs["bo"], f64) + bv_f @ Wo).astype(np.float32)
    bf1_f = (np.asarray(inputs["bf1"], f64) + b2 @ W1).astype(np.float32)

    weights = {
        "Wq": np.ascontiguousarray(Wq_f), "Wk": np.ascontiguousarray(Wk_f),
        "Wv": np.ascontiguousarray(Wv_f),
        "Wo": np.ascontiguousarray(np.asarray(inputs["Wo"], np.float32)),
        "W1": np.ascontiguousarray(W1_f),
        "W2": np.ascontiguousarray(np.asarray(inputs["W2"], np.float32)),
        "bq": bq_f, "bk": bk_f, "bo2": bo2, "bf1": bf1_f,
        "bf2": np.ascontiguousarray(np.asarray(inputs["bf2"], np.float32)),
    }

    if "nc" not in _NC_CACHE:
        _NC_CACHE["nc"] = _build_nc()
    nc = _NC_CACHE["nc"]

    in_maps = []
    for c in range(N_CORES):
        b, qc = c // 4, c % 4
        xb = np.roll(x[b], -qc * SQ, axis=0)
        m = {"xp": np.ascontiguousarray(xb)}
        m.update(weights)
        in_maps.append(m)

    res = bass_utils.run_bass_kernel_spmd(nc, in_maps, core_ids=list(range(N_CORES)))
    out = np.empty((B, S, H), dtype=np.float32)
    for c in range(N_CORES):
        b, qc = c // 4, c % 4
        out[b, qc * SQ:(qc + 1) * SQ, :] = res.results[c]["y"]
    return out


# revision 23
# speedup vs baseline: 1.0165x; 1.0154x over previous
"""Trainium2 Bass kernel for a dense transformer AttentionBlock (optimized v2).

Problem: x[2,2048,1024] fp32 -> LN1 -> MHA(16 heads, hd=64) + residual
         -> LN2 -> FFN(4096, relu) + residual.

Sharding: 8-way data parallel. Core c handles batch b=c//4 and query chunk
qc=c%4 (512 tokens). K/V are computed for the full 2048-token sequence on
each core (redundant within a batch group, but no collectives needed).
The host rotates each core's token axis so its query chunk is rows 0:512 —
softmax over keys is permutation invariant so key order does not matter.

Host-side exact weight algebra (one-time O(H^2) prep, keeps device math
identical): LN affines are folded into the projection weights
(Wq' = g1*Wq rows, bq' = bq + b1@Wq, same for K/V; W1' = g2*W1,
bf1' = bf1 + b2@W1) and the V bias is folded through the output projection
(bo2 = bo + bv'@Wo, exact because softmax rows sum to 1). Wq/Wk/Wv ship
pre-cast to bf16 (the attention matmul dtype) to halve their DMA bytes.

Precision: FFN / output-proj matmuls in float32r (full PE rate at N>=512);
attention datapath bf16 with fp32 PSUM. Softmax uses exp without max
subtraction; denominator accumulated via a ones-column in the V operand and
divided out per head (PE broadcast of the reciprocal row).

Scheduling notes (tuned against the TimelineSim cost model): all DMA
transfers serialize on one ~360 GB/s pipe, so x tiles are dispatched ahead
of Wv; Wo / x-residual / first W1 tiles live in pools opened before phase 1
so their SP-queue DMAs (emitted after phase 1) stream during attention; LN
runs two tiles ahead of the transpose+V work so PE never idles (idle gaps
cost 2-3.7x p-state matmul penalties for the next ~3us).
"""

import numpy as np
from contextlib import ExitStack

B, S, H = 2, 2048, 1024
NH, HD = 16, 64
FF = 4 * H
EPS = 1e-5
P = 128
SQ = 512          # query-chunk tokens per core
N_CORES = 8
TT_Q = SQ // P    # 4 token tiles in the query chunk
TT_S = S // P     # 16 token tiles in the full sequence
KH = H // P       # 8 k-tiles over hidden dim


def _build_nc():
    import concourse.bass as bass
    import concourse.mybir as mybir
    import concourse.tile as tile
    from concourse import bacc
    from concourse.masks import make_identity

    dt = mybir.dt
    f32 = dt.float32
    f32r = dt.float32r
    bf16 = dt.bfloat16
    AF = mybir.ActivationFunctionType
    ALU = mybir.AluOpType

    nc = bacc.Bacc(None, target_bir_lowering=False)

    xp = nc.dram_tensor("xp", [S, H], f32, kind="ExternalInput")
    Wq = nc.dram_tensor("Wq", [H, H], bf16, kind="ExternalInput")
    Wk = nc.dram_tensor("Wk", [H, H], bf16, kind="ExternalInput")
    Wv = nc.dram_tensor("Wv", [H, H], bf16, kind="ExternalInput")
    Wo = nc.dram_tensor("Wo", [H, H], f32, kind="ExternalInput")
    W1 = nc.dram_tensor("W1", [H, FF], f32, kind="ExternalInput")
    W2 = nc.dram_tensor("W2", [FF, H], f32, kind="ExternalInput")
    bq = nc.dram_tensor("bq", [H], f32, kind="ExternalInput")
    bk = nc.dram_tensor("bk", [H], f32, kind="ExternalInput")
    bo2 = nc.dram_tensor("bo2", [H], f32, kind="ExternalInput")
    bf1 = nc.dram_tensor("bf1", [FF], f32, kind="ExternalInput")
    bf2 = nc.dram_tensor("bf2", [H], f32, kind="ExternalInput")
    y = nc.dram_tensor("y", [SQ, H], f32, kind="ExternalOutput")

    def col_view(dram_vec, n):
        # DRAM [n*P] viewed as [P, n]: element [p, j] = vec[j*P + p]
        return dram_vec.rearrange("(a p) -> p a", p=P)

    def layernorm_tile(pool, src_ap, tag, xn_dt=f32, tc=None):
        """token-major LN of one [P, H] tile -> (x - m) * rstd. The per
        feature g/b affine is folded into the weights on the host. The tiny
        scalar-chain ops run at high scheduler priority so they are not
        deferred behind the next tile's bulk stats (latency-critical)."""
        from contextlib import nullcontext
        hp = tc.high_priority() if tc is not None else nullcontext()
        stats = pool.tile([P, 2, 6], f32, tag=tag + "st")
        nc.vector.bn_stats(out=stats[:, 0, :], in_=src_ap[:, 0:512])
        nc.vector.bn_stats(out=stats[:, 1, :], in_=src_ap[:, 512:1024])
        mv = pool.tile([P, 2], f32, tag=tag + "mv")
        rstd = pool.tile([P, 1], f32, tag=tag + "rs")
        negmr = pool.tile([P, 1], f32, tag=tag + "nm")
        with hp:
            nc.vector.bn_aggr(out=mv, in_=stats)
            nc.scalar.activation(out=rstd, in_=mv[:, 1:2], func=AF.Sqrt,
                                 bias=eps_t, scale=1.0)
            nc.vector.reciprocal(out=rstd, in_=rstd)
            nc.vector.scalar_tensor_tensor(
                out=negmr, in0=mv[:, 0:1], scalar=rstd, in1=negone,
                op0=ALU.mult, op1=ALU.mult)
        xn = pool.tile([P, H], xn_dt, tag=tag + "xn")
        nc.scalar.activation(out=xn, in_=src_ap, func=AF.Identity,
                             bias=negmr, scale=rstd)
        return xn

    with tile.TileContext(nc) as tc:
        with (
            tc.tile_pool(name="consts", bufs=1) as consts,
            tc.tile_pool(name="persistA", bufs=1) as persistA,
        ):
            ctxT = persistA.tile([P, KH, SQ], f32r)      # 16 KB/part

            ident_b = consts.tile([P, P], bf16)
            make_identity(nc, ident_b)
            eps_t = consts.tile([P, 1], f32)
            nc.vector.memset(eps_t, EPS)
            negone = consts.tile([P, 1], f32)
            nc.vector.memset(negone, -1.0)
            ident_f = consts.tile([P, P], f32)
            make_identity(nc, ident_f)
            ident_r = consts.tile([P, P], f32r)
            nc.scalar.copy(out=ident_r, in_=ident_f)
            # f32r constant 1.0 (memset can't emit f32r; ACT Copy(0*x+1) can;
            # the input is ignored at scale=0)
            ones_t = consts.tile([1, HD], f32r)
            nc.scalar.activation(out=ones_t, in_=ident_f[0:1, 0:HD], func=AF.Copy,
                                 bias=1.0, scale=0.0)

            # persistB (phase-3+ accumulators) and the prefetch pools open
            # BEFORE phase 1 so they get SBUF space disjoint from the
            # phase-1/2 pools: their SP-queue DMAs (emitted after the
            # phase-1 loop) then stream during attention with no space-reuse
            # dependencies. prefetch_es closes right after phase 3 (LIFO) to
            # make room for the FFN pools.
            persistB_es = ExitStack()
            persistB = persistB_es.enter_context(tc.tile_pool(name="persistB", bufs=1))
            out_res = persistB.tile([P, TT_Q, H], f32)   # 16 KB/part
            stats2 = persistB.tile([P, TT_Q, KH, 6], f32)

            w1p_es = ExitStack()
            w1p = w1p_es.enter_context(tc.tile_pool(name="w1p", bufs=4))

            prefetch_es = ExitStack()
            wop = prefetch_es.enter_context(tc.tile_pool(name="wo", bufs=1))
            wqkv = prefetch_es.enter_context(tc.tile_pool(name="wqkv", bufs=2))
            qtp = prefetch_es.enter_context(tc.tile_pool(name="qt", bufs=2))

            with tc.tile_pool(name="xnTp", bufs=1) as xnTp:
                xnTc = [xnTp.tile([P, KH, SQ], bf16, tag=f"xnt{c}", name=f"xnt{c}")
                        for c in range(4)]               # 4 x 8 KB/part

                with tc.tile_pool(name="vap", bufs=1) as vap:
                    # token-major V (+ ones column for softmax denominator)
                    va = vap.tile([P, TT_S, NH, HD + 1], bf16)
                    nc.vector.memset(va[:, :, :, HD:HD + 1], 1.0)

                    # ------- Phase 1: LN1 + transpose + V projection -------
                    with (
                        tc.tile_pool(name="ln", bufs=4) as ln,
                        tc.tile_pool(name="lnst", bufs=4) as lnst,
                        tc.tile_pool(name="wvp", bufs=1) as wvp,
                        tc.tile_pool(name="ps_t", bufs=2, space="PSUM") as ps_t,
                        tc.tile_pool(name="ps_v", bufs=2, space="PSUM") as ps_v,
                    ):
                        # LN is software-pipelined two tiles ahead of the
                        # transpose/V work so PE never waits on the LN chain.
                        xns = {}

                        def emit_ln(i):
                            xt = ln.tile([P, H], f32, tag="xt", name="xt")
                            if i < 3:
                                # halves so LN stats start ~1.5us sooner on
                                # the pipeline-fill tiles
                                nc.sync.dma_start(out=xt[:, 0:512],
                                                  in_=xp[i * P:(i + 1) * P, 0:512])
                                nc.sync.dma_start(out=xt[:, 512:1024],
                                                  in_=xp[i * P:(i + 1) * P, 512:1024])
                            else:
                                nc.sync.dma_start(out=xt, in_=xp[i * P:(i + 1) * P, :])
                            xns[i] = layernorm_tile(lnst, xt, "l1", xn_dt=bf16, tc=tc)

                        emit_ln(0)
                        emit_ln(1)
                        emit_ln(2)

                        # Wv quarters queue behind the first x tiles on the
                        # (serial) DMA pipe.
                        wv = wvp.tile([P, KH, H], bf16)
                        wv_view = Wv.rearrange("(a p) c -> p a c", p=P)
                        nc.scalar.dma_start(out=wv[:, :, 0:256], in_=wv_view[:, :, 0:256])
                        nc.gpsimd.dma_start(out=wv[:, :, 256:512], in_=wv_view[:, :, 256:512])
                        nc.scalar.dma_start(out=wv[:, :, 512:768], in_=wv_view[:, :, 512:768])
                        nc.gpsimd.dma_start(out=wv[:, :, 768:1024], in_=wv_view[:, :, 768:1024])

                        bq_t = consts.tile([P, KH], f32)
                        bk_t = consts.tile([P, KH], f32)
                        bo2_t = consts.tile([P, KH], f32)
                        bf2_t = consts.tile([P, KH], f32)
                        bf1_t = consts.tile([P, FF // P], f32)
                        nc.gpsimd.dma_start(out=bq_t, in_=col_view(bq, KH))
                        nc.gpsimd.dma_start(out=bk_t, in_=col_view(bk, KH))
                        nc.gpsimd.dma_start(out=bo2_t, in_=col_view(bo2, KH))
                        nc.gpsimd.dma_start(out=bf2_t, in_=col_view(bf2, KH))
                        nc.gpsimd.dma_start(out=bf1_t, in_=col_view(bf1, FF // P))

                        def emit_v(i):
                            # V projection for tile i (runs one tile behind
                            # the transposes so PE never waits on the
                            # PSUM->SBUF evacuation of xnT)
                            for fg in range(2):
                                psw = ps_v.tile([P, SQ], f32, tag="psv",
                                                name="psw")
                                for ks in range(KH):
                                    nc.tensor.matmul(
                                        psw, xnTc[i // 4][:, ks, (i % 4) * P:(i % 4 + 1) * P],
                                        wv[:, ks, fg * SQ:(fg + 1) * SQ],
                                        start=(ks == 0), stop=(ks == KH - 1))
                                dstv = va[:, i, 8 * fg:8 * fg + 8, 0:HD]
                                src = psw.rearrange("p (h d) -> p h d", d=HD)
                                if fg == 0:
                                    nc.vector.tensor_copy(out=dstv, in_=src)
                                else:
                                    nc.scalar.copy(out=dstv, in_=src)

                        def emit_q0():
                            # Q projection for head-group 0 (only needs
                            # xnTc[0], ready after tile 3) — fills the PE
                            # starvation window while LN1 still paces tiles.
                            wk4 = wqkv.tile([P, KH, 256], bf16, tag="wk4",
                                            name="wk4")
                            wq4 = wqkv.tile([P, KH, 256], bf16, tag="wq4",
                                            name="wq4")
                            nc.gpsimd.dma_start(
                                out=wk4,
                                in_=Wk.rearrange("(a p) c -> p a c", p=P)[:, :, 0:256])
                            nc.gpsimd.dma_start(
                                out=wq4,
                                in_=Wq.rearrange("(a p) c -> p a c", p=P)[:, :, 0:256])
                            QTp = []
                            for pair in range(2):
                                ps = ps_v.tile([P, SQ], f32, tag="psv",
                                               name="psq0")
                                for ks in range(KH):
                                    nc.tensor.matmul(
                                        ps, wq4[:, ks, pair * P:(pair + 1) * P],
                                        xnTc[0][:, ks, :],
                                        start=(ks == 0), stop=(ks == KH - 1))
                                qt0 = qtp.tile([P, SQ], bf16, tag=f"qtp{pair}",
                                               name=f"qtp{pair}")
                                nc.vector.tensor_scalar_add(
                                    out=qt0, in0=ps, scalar1=bq_t[:, pair:pair + 1])
                                QTp.append(qt0)
                            return wk4, wq4, QTp


                        g0_hoist = {}
                        for i in range(TT_S):
                            if i + 3 < TT_S:
                                emit_ln(i + 3)
                            if i == 4:
                                g0_hoist["v"] = emit_q0()
                            xn = xns.pop(i)
                            for j2 in range(2):
                                ps = ps_t.tile([P, 4, P], bf16, tag="pst")
                                for k in range(4):
                                    j = 4 * j2 + k
                                    nc.tensor.transpose(
                                        ps[:, k, :], xn[:, j * P:(j + 1) * P], ident_b)
                                dst = xnTc[i // 4][:, 4 * j2:4 * j2 + 4,
                                                  (i % 4) * P:(i % 4 + 1) * P]
                                if j2 == 0:
                                    nc.vector.tensor_copy(out=dst, in_=ps)
                                else:
                                    nc.scalar.copy(out=dst, in_=ps)
                            if i >= 1:
                                emit_v(i - 1)
                        emit_v(TT_S - 1)

                    # Prefetch DMAs for phase 3+ (SP queue drains these
                    # during attention; pools were opened before phase 1).
                    wo_ts = []
                    for os_ in range(KH):
                        wo_t = wop.tile([P, KH, P], f32r, tag=f"wo{os_}",
                                        name=f"wo{os_}")
                        nc.sync.dma_start(
                            out=wo_t,
                            in_=Wo.rearrange("(a p) c -> p a c", p=P)[:, :, os_ * P:(os_ + 1) * P].bitcast(f32r))
                        wo_ts.append(wo_t)
                    w1_ts = {}
                    for ft in range(4):
                        w1_t = w1p.tile([P, KH, P], f32r, tag="w1_t", name="w1_t")
                        nc.sync.dma_start(
                            out=w1_t,
                            in_=W1.rearrange("(a p) c -> p a c", p=P)[:, :, ft * P:(ft + 1) * P].bitcast(f32r))
                        w1_ts[ft] = w1_t

                    # ---------------- Phase 2: K/Q + attention ----------------
                    with ExitStack() as es2:
                        ktp = es2.enter_context(tc.tile_pool(name="kt", bufs=2))
                        pexp = es2.enter_context(tc.tile_pool(name="pexp", bufs=6))
                        attn_sm = es2.enter_context(tc.tile_pool(name="attn_sm", bufs=2))
                        ps_qkv = es2.enter_context(tc.tile_pool(name="ps_qkv", bufs=2, space="PSUM"))
                        ps_s = es2.enter_context(tc.tile_pool(name="ps_s", bufs=3, space="PSUM"))
                        ps_ctx = es2.enter_context(tc.tile_pool(name="ps_ctx", bufs=1, space="PSUM"))
                        ps_b = es2.enter_context(tc.tile_pool(name="ps_b", bufs=1, space="PSUM"))
                        for g4 in range(4):
                            c0 = g4 * 4 * HD  # first feature column of this group
                            if g4 == 0:
                                wk4, wq4, QTp = g0_hoist.pop("v")
                            else:
                                wk4 = wqkv.tile([P, KH, 256], bf16, tag="wk4")
                                wq4 = wqkv.tile([P, KH, 256], bf16, tag="wq4")
                                nc.gpsimd.dma_start(
                                    out=wk4,
                                    in_=Wk.rearrange("(a p) c -> p a c", p=P)[:, :, c0:c0 + 256])
                                nc.gpsimd.dma_start(
                                    out=wq4,
                                    in_=Wq.rearrange("(a p) c -> p a c", p=P)[:, :, c0:c0 + 256])

                                # Q^T pair-stacked: [128(2 heads), SQ] per pair
                                QTp = [qtp.tile([P, SQ], bf16, tag=f"qtp{pair}",
                                                name=f"qtp{pair}") for pair in range(2)]
                                for pair in range(2):
                                    hp = 2 * g4 + pair
                                    ps = ps_qkv.tile([P, SQ], f32, tag="psqkv")
                                    for ks in range(KH):
                                        nc.tensor.matmul(
                                            ps, wq4[:, ks, pair * P:(pair + 1) * P],
                                            xnTc[0][:, ks, :],
                                            start=(ks == 0), stop=(ks == KH - 1))
                                    nc.vector.tensor_scalar_add(
                                        out=QTp[pair], in0=ps,
                                        scalar1=bq_t[:, hp:hp + 1])

                            for pair in range(2):
                                hp = 2 * g4 + pair
                                # K^T pair-stacked: [128(2 heads), S]
                                KTp = ktp.tile([P, S], bf16, tag="ktp")
                                for t4 in range(4):
                                    ps = ps_qkv.tile([P, SQ], f32, tag="psqkv")
                                    for ks in range(KH):
                                        nc.tensor.matmul(
                                            ps, wk4[:, ks, pair * P:(pair + 1) * P],
                                            xnTc[t4][:, ks, :],
                                            start=(ks == 0), stop=(ks == KH - 1))
                                    nc.vector.tensor_scalar_add(
                                        out=KTp[:, t4 * SQ:(t4 + 1) * SQ], in0=ps,
                                        scalar1=bk_t[:, hp:hp + 1])

                                # both heads of the pair interleaved per key
                                # chunk (disjoint PE row groups 0/64).
                                cpss = [ps_ctx.tile([HD + 1, SQ], f32,
                                                    tag=f"ctxps{rh}",
                                                    name=f"ctxps{rh}")
                                        for rh in range(2)]
                                for kc in range(TT_S):
                                    pts = []
                                    for rh in range(2):
                                        rb = rh * HD
                                        sps = ps_s.tile([P, SQ], f32, tag="sps")
                                        nc.tensor.matmul(
                                            sps, KTp[rb:rb + HD, kc * P:(kc + 1) * P],
                                            QTp[pair][rb:rb + HD, :],
                                            start=True, stop=True)
                                        pt = pexp.tile([P, SQ], bf16, tag="pt")
                                        nc.scalar.activation(
                                            out=pt, in_=sps, func=AF.Exp,
                                            scale=float(1.0 / np.sqrt(HD)))
                                        pts.append(pt)
                                    for rh in range(2):
                                        hh = 2 * pair + rh
                                        nc.tensor.matmul(
                                            cpss[rh], va[:, kc, 4 * g4 + hh, :], pts[rh],
                                            start=(kc == 0), stop=(kc == TT_S - 1))
                                for rh in range(2):
                                    hh = 2 * pair + rh   # head within group
                                    h = 4 * g4 + hh      # global head
                                    cps = cpss[rh]
                                    # ctx[0:64,:] / l  (l = cps[64,:])
                                    rl = attn_sm.tile([1, SQ], f32r, tag="rl")
                                    with nc.allow_low_precision(reason="softmax denom bcast"):
                                        nc.vector.reciprocal(out=rl, in_=cps[HD:HD + 1, :])
                                    # PE broadcast of 1/l to 64 rows
                                    bps = ps_b.tile([HD, SQ], f32, tag="bps")
                                    nc.tensor.matmul(bps, ones_t, rl,
                                                     start=True, stop=True)
                                    bsb = attn_sm.tile([HD, SQ], f32, tag="bsb")
                                    nc.vector.tensor_copy(out=bsb, in_=bps)
                                    slot = h // 2
                                    dst = ctxT[rh * HD:(rh + 1) * HD, slot, :]
                                    nc.vector.tensor_mul(dst, cps[0:HD, :], bsb)

            # xnT freed here.
            if True:
                # ---------------- Phase 3: output proj + residual ----------
                with (
                    tc.tile_pool(name="xrp", bufs=1) as xrp,
                    tc.tile_pool(name="ot", bufs=3) as otp,
                    tc.tile_pool(name="ps_o", bufs=3, space="PSUM") as ps_o,
                    tc.tile_pool(name="ps_t3", bufs=2, space="PSUM") as ps_t3,
                ):
                    xr = xrp.tile([P, TT_Q, H], f32)
                    for tt in range(TT_Q):
                        nc.sync.dma_start(out=xr[:, tt, :],
                                          in_=xp[tt * P:(tt + 1) * P, :])
                    oTs = {}

                    def emit_o_tail(os_):
                        # transposes + residual add + LN2 stats, one os_
                        # behind the matmuls so PE never waits on the ACT
                        # PSUM evacuation. The last os_ is latency-critical
                        # (it gates the whole LN2->FFN chain), so its add is
                        # split per tile to overlap with the stats.
                        oT = oTs.pop(os_)
                        pst = ps_t3.tile([P, TT_Q, P], f32r, tag="pst3")
                        for tt in range(TT_Q):
                            nc.tensor.transpose(pst[:, tt, :],
                                                oT[:, tt * P:(tt + 1) * P], ident_r)
                        nc.vector.tensor_add(
                            out_res[:, :, os_ * P:(os_ + 1) * P], pst,
                            xr[:, :, os_ * P:(os_ + 1) * P])
                        for tt in range(TT_Q):
                            nc.vector.bn_stats(
                                out=stats2[:, tt, os_, :],
                                in_=out_res[:, tt, os_ * P:(os_ + 1) * P])

                    for os_ in range(KH):
                        ps = ps_o.tile([P, SQ], f32, tag="pso")
                        for cs in range(KH):
                            nc.tensor.matmul(ps, wo_ts[os_][:, cs, :], ctxT[:, cs, :],
                                             start=(cs == 0), stop=(cs == KH - 1))
                        oT = otp.tile([P, SQ], f32r, tag="oT")
                        nc.scalar.activation(out=oT, in_=ps, func=AF.Identity,
                                             bias=bo2_t[:, os_:os_ + 1], scale=1.0)
                        oTs[os_] = oT
                        if os_ >= 1:
                            emit_o_tail(os_ - 1)
                    emit_o_tail(KH - 1)

                # release the wo/wqkv prefetch space (LIFO top); xn2T takes
                # that space for the FFN.
                prefetch_es.close()
                w1xp_es = ExitStack()
                w1xp = w1xp_es.enter_context(tc.tile_pool(name="w1xp", bufs=1))
                xn2T = w1xp.tile([P, KH, SQ], f32r)          # 16 KB/part

                # ---------------- Phase 4: LN2 -> xn2^T ----------------
                with (
                    tc.tile_pool(name="lnst2", bufs=4) as lnst2,
                    tc.tile_pool(name="ps_t4", bufs=2, space="PSUM") as ps_t4,
                ):
                    # batched scalar chain for all 4 tiles (one hop each),
                    # high priority: this latency gates the whole FFN
                    mv4 = lnst2.tile([P, TT_Q, 2], f32)
                    rstd4 = lnst2.tile([P, TT_Q], f32)
                    negmr4 = lnst2.tile([P, TT_Q], f32)
                    with tc.high_priority():
                        for i in range(TT_Q):
                            nc.vector.bn_aggr(out=mv4[:, i, :], in_=stats2[:, i, :, :])
                        nc.scalar.activation(out=rstd4, in_=mv4[:, :, 1], func=AF.Sqrt,
                                             bias=eps_t, scale=1.0)
                        nc.vector.reciprocal(out=rstd4, in_=rstd4)
                        nc.vector.scalar_tensor_tensor(
                            out=negmr4, in0=mv4[:, :, 0], scalar=negone, in1=rstd4,
                            op0=ALU.mult, op1=ALU.mult)
                    for i in range(TT_Q):
                        xn2 = lnst2.tile([P, H], f32r, tag="xn2")
                        for hh2 in range(2):
                            nc.scalar.activation(
                                out=xn2[:, hh2 * 512:(hh2 + 1) * 512],
                                in_=out_res[:, i, hh2 * 512:(hh2 + 1) * 512],
                                func=AF.Identity,
                                bias=negmr4[:, i:i + 1], scale=rstd4[:, i:i + 1])
                        for j2 in range(2):
                            ps = ps_t4.tile([P, 4, P], f32r, tag="pst4")
                            for k in range(4):
                                j = 4 * j2 + k
                                nc.tensor.transpose(
                                    ps[:, k, :], xn2[:, j * P:(j + 1) * P], ident_r)
                            dst = xn2T[:, 4 * j2:4 * j2 + 4, i * P:(i + 1) * P]
                            nc.vector.tensor_copy(out=dst, in_=ps)

                # ---------------- Phase 5: FFN ----------------
                with tc.tile_pool(name="hTp", bufs=1) as hTp:
                    hT = hTp.tile([P, FF // P, SQ], f32r)   # 64 KB/part
                    with (
                        tc.tile_pool(name="w2p", bufs=6) as w2p,
                        tc.tile_pool(name="ps_f", bufs=3, space="PSUM") as ps_f,
                    ):
                        for ft in range(FF // P):
                            if ft in w1_ts:
                                w1_t = w1_ts.pop(ft)
                            else:
                                w1_t = w1p.tile([P, KH, P], f32r, tag="w1_t",
                                                name="w1_t")
                                nc.sync.dma_start(
                                    out=w1_t,
                                    in_=W1.rearrange("(a p) c -> p a c", p=P)[:, :, ft * P:(ft + 1) * P].bitcast(f32r))
                            ps = ps_f.tile([P, SQ], f32, tag="psf")
                            for ks in range(KH):
                                nc.tensor.matmul(ps, w1_t[:, ks, :], xn2T[:, ks, :],
                                                 start=(ks == 0), stop=(ks == KH - 1))
                            nc.scalar.activation(out=hT[:, ft, :], in_=ps, func=AF.Relu,
                                                 bias=bf1_t[:, ft:ft + 1], scale=1.0)

                        with (
                            tc.tile_pool(name="o2", bufs=3) as o2p,
                            tc.tile_pool(name="yout", bufs=2) as youtp,
                            tc.tile_pool(name="ps_o2", bufs=2, space="PSUM") as ps_o2,
                            tc.tile_pool(name="ps_t5", bufs=2, space="PSUM") as ps_t5,
                        ):
                            o2Ts = {}

                            def emit_y_tail(os_):
                                o2T = o2Ts.pop(os_)
                                pst = ps_t5.tile([P, TT_Q, P], f32r, tag="pst5")
                                for tt in range(TT_Q):
                                    nc.tensor.transpose(pst[:, tt, :],
                                                        o2T[:, tt * P:(tt + 1) * P], ident_r)
                                yo = youtp.tile([P, TT_Q, P], f32, tag="yo")
                                nc.vector.tensor_add(
                                    yo, pst, out_res[:, :, os_ * P:(os_ + 1) * P])
                                nc.sync.dma_start(
                                    out=y.rearrange("(t p) c -> p t c", p=P)[:, :, os_ * P:(os_ + 1) * P],
                                    in_=yo)

                            for os_ in range(KH):
                                ps = ps_o2.tile([P, SQ], f32, tag="pso2")
                                for q4 in range(4):
                                    w2_t = w2p.tile([P, 8, P], f32r, tag="w2_t")
                                    dma_eng = nc.gpsimd if q4 % 2 == 0 else nc.sync
                                    dma_eng.dma_start(
                                        out=w2_t,
                                        in_=W2.rearrange("(a p) c -> p a c", p=P)[:, q4 * 8:(q4 + 1) * 8, os_ * P:(os_ + 1) * P].bitcast(f32r))
                                    for f8 in range(8):
                                        ft = q4 * 8 + f8
                                        nc.tensor.matmul(ps, w2_t[:, f8, :], hT[:, ft, :],
                                                         start=(ft == 0), stop=(ft == FF // P - 1))
                                o2T = o2p.tile([P, SQ], f32r, tag="o2T")
                                nc.scalar.activation(out=o2T, in_=ps, func=AF.Identity,
                                                     bias=bf2_t[:, os_:os_ + 1], scale=1.0)
                                o2Ts[os_] = o2T
                                if os_ >= 1:
                                    emit_y_tail(os_ - 1)
                            emit_y_tail(KH - 1)

                # LIFO pool teardown for the manually-entered pools
                w1xp_es.close()
                prefetch_es2 = None  # (placeholder, nothing else open here)
                w1p_es.close()
                persistB_es.close()

    nc.finalize()
    return nc


_NC_CACHE = {}


def _prepare_in_maps(inputs):
    """Host-side weight algebra + per-core sharding (see module docstring).
    Returns the per-core input maps for run_bass_kernel_spmd."""
    import ml_dtypes

    x = np.ascontiguousarray(np.asarray(inputs["x"], dtype=np.float32))
    f64 = np.float64
    g1 = np.asarray(inputs["g1"], f64)
    b1 = np.asarray(inputs["b1"], f64)
    g2 = np.asarray(inputs["g2"], f64)
    b2 = np.asarray(inputs["b2"], f64)
    Wq = np.asarray(inputs["Wq"], f64)
    Wk = np.asarray(inputs["Wk"], f64)
    Wv = np.asarray(inputs["Wv"], f64)
    Wo = np.asarray(inputs["Wo"], f64)
    W1 = np.asarray(inputs["W1"], f64)

    bf16 = ml_dtypes.bfloat16
    # exact affine folds (see module docstring)
    Wq_f = (g1[:, None] * Wq).astype(np.float32).astype(bf16)
    Wk_f = (g1[:, None] * Wk).astype(np.float32).astype(bf16)
    Wv_f = (g1[:, None] * Wv).astype(np.float32).astype(bf16)
    W1_f = (g2[:, None] * W1).astype(np.float32)
    bq_f = (np.asarray(inputs["bq"], f64) + b1 @ Wq).astype(np.float32)
    bk_f = (np.asarray(inputs["bk"], f64) + b1 @ Wk).astype(np.float32)
    bv_f = np.asarray(inputs["bv"], f64) + b1 @ Wv
    bo2 = (np.asarray(inputs["bo"], f64) + bv_f @ Wo).astype(np.float32)
    bf1_f = (np.asarray(inputs["bf1"], f64) + b2 @ W1).astype(np.float32)

    weights = {
        "Wq": np.ascontiguousarray(Wq_f), "Wk": np.ascontiguousarray(Wk_f),
        "Wv": np.ascontiguousarray(Wv_f),
        "Wo": np.ascontiguousarray(np.asarray(inputs["Wo"], np.float32)),
        "W1": np.ascontiguousarray(W1_f),
        "W2": np.ascontiguousarray(np.asarray(inputs["W2"], np.float32)),
        "bq": bq_f, "bk": bk_f, "bo2": bo2, "bf1": bf1_f,
        "bf2": np.ascontiguousarray(np.asarray(inputs["bf2"], np.float32)),
    }

    in_maps = []
    for c in range(N_CORES):
        b, qc = c // 4, c % 4
        xb = np.roll(x[b], -qc * SQ, axis=0)
        m = {"xp": np.ascontiguousarray(xb)}
        m.update(weights)
        in_maps.append(m)
    return in_maps


def kernel(**inputs):
    import concourse.bass_utils as bass_utils

    if "nc" not in _NC_CACHE:
        _NC_CACHE["nc"] = _build_nc()
    nc = _NC_CACHE["nc"]
    in_maps = _prepare_in_maps(inputs)
    res = bass_utils.run_bass_kernel_spmd(nc, in_maps, core_ids=list(range(N_CORES)))
    out = np.empty((B, S, H), dtype=np.float32)
    for c in range(N_CORES):
        b, qc = c // 4, c % 4
        out[b, qc * SQ:(qc + 1) * SQ, :] = res.results[c]["y"]
    return out


# revision 24
# speedup vs baseline: 1.0203x; 1.0037x over previous
"""Trainium2 Bass kernel for a dense transformer AttentionBlock (optimized v2).

Problem: x[2,2048,1024] fp32 -> LN1 -> MHA(16 heads, hd=64) + residual
         -> LN2 -> FFN(4096, relu) + residual.

Sharding: 8-way data parallel. Core c handles batch b=c//4 and query chunk
qc=c%4 (512 tokens). K/V are computed for the full 2048-token sequence on
each core (redundant within a batch group, but no collectives needed).
The host rotates each core's token axis so its query chunk is rows 0:512 —
softmax over keys is permutation invariant so key order does not matter.

Host-side exact weight algebra (one-time O(H^2) prep, keeps device math
identical): LN affines are folded into the projection weights
(Wq' = g1*Wq rows, bq' = bq + b1@Wq, same for K/V; W1' = g2*W1,
bf1' = bf1 + b2@W1) and the V bias is folded through the output projection
(bo2 = bo + bv'@Wo, exact because softmax rows sum to 1). Wq/Wk/Wv ship
pre-cast to bf16 (the attention matmul dtype) to halve their DMA bytes.

Precision: FFN / output-proj matmuls in float32r (full PE rate at N>=512);
attention datapath bf16 with fp32 PSUM. Softmax uses exp without max
subtraction; denominator accumulated via a ones-column in the V operand and
divided out per head (PE broadcast of the reciprocal row).

Scheduling notes (tuned against the TimelineSim cost model): all DMA
transfers serialize on one ~360 GB/s pipe, so x tiles are dispatched ahead
of Wv; Wo / x-residual / first W1 tiles live in pools opened before phase 1
so their SP-queue DMAs (emitted after phase 1) stream during attention; LN
runs two tiles ahead of the transpose+V work so PE never idles (idle gaps
cost 2-3.7x p-state matmul penalties for the next ~3us).
"""

import numpy as np
from contextlib import ExitStack

B, S, H = 2, 2048, 1024
NH, HD = 16, 64
FF = 4 * H
EPS = 1e-5
P = 128
SQ = 512          # query-chunk tokens per core
N_CORES = 8
TT_Q = SQ // P    # 4 token tiles in the query chunk
TT_S = S // P     # 16 token tiles in the full sequence
KH = H // P       # 8 k-tiles over hidden dim


def _build_nc():
    import concourse.bass as bass
    import concourse.mybir as mybir
    import concourse.tile as tile
    from concourse import bacc
    from concourse.masks import make_identity

    dt = mybir.dt
    f32 = dt.float32
    f32r = dt.float32r
    bf16 = dt.bfloat16
    AF = mybir.ActivationFunctionType
    ALU = mybir.AluOpType

    nc = bacc.Bacc(None, target_bir_lowering=False)

    xp = nc.dram_tensor("xp", [S, H], f32, kind="ExternalInput")
    Wq = nc.dram_tensor("Wq", [H, H], bf16, kind="ExternalInput")
    Wk = nc.dram_tensor("Wk", [H, H], bf16, kind="ExternalInput")
    Wv = nc.dram_tensor("Wv", [H, H], bf16, kind="ExternalInput")
    Wo = nc.dram_tensor("Wo", [H, H], f32, kind="ExternalInput")
    W1 = nc.dram_tensor("W1", [H, FF], f32, kind="ExternalInput")
    W2 = nc.dram_tensor("W2", [FF, H], f32, kind="ExternalInput")
    bq = nc.dram_tensor("bq", [H], f32, kind="ExternalInput")
    bk = nc.dram_tensor("bk", [H], f32, kind="ExternalInput")
    bo2 = nc.dram_tensor("bo2", [H], f32, kind="ExternalInput")
    bf1 = nc.dram_tensor("bf1", [FF], f32, kind="ExternalInput")
    bf2 = nc.dram_tensor("bf2", [H], f32, kind="ExternalInput")
    y = nc.dram_tensor("y", [SQ, H], f32, kind="ExternalOutput")

    def col_view(dram_vec, n):
        # DRAM [n*P] viewed as [P, n]: element [p, j] = vec[j*P + p]
        return dram_vec.rearrange("(a p) -> p a", p=P)

    def layernorm_tile(pool, src_ap, tag, xn_dt=f32, tc=None):
        """token-major LN of one [P, H] tile -> (x - m) * rstd. The per
        feature g/b affine is folded into the weights on the host. The tiny
        scalar-chain ops run at high scheduler priority so they are not
        deferred behind the next tile's bulk stats (latency-critical)."""
        from contextlib import nullcontext
        hp = tc.high_priority() if tc is not None else nullcontext()
        stats = pool.tile([P, 2, 6], f32, tag=tag + "st")
        nc.vector.bn_stats(out=stats[:, 0, :], in_=src_ap[:, 0:512])
        nc.vector.bn_stats(out=stats[:, 1, :], in_=src_ap[:, 512:1024])
        mv = pool.tile([P, 2], f32, tag=tag + "mv")
        rstd = pool.tile([P, 1], f32, tag=tag + "rs")
        negmr = pool.tile([P, 1], f32, tag=tag + "nm")
        with hp:
            nc.vector.bn_aggr(out=mv, in_=stats)
            nc.scalar.activation(out=rstd, in_=mv[:, 1:2], func=AF.Sqrt,
                                 bias=eps_t, scale=1.0)
            nc.vector.reciprocal(out=rstd, in_=rstd)
            nc.vector.scalar_tensor_tensor(
                out=negmr, in0=mv[:, 0:1], scalar=rstd, in1=negone,
                op0=ALU.mult, op1=ALU.mult)
        xn = pool.tile([P, H], xn_dt, tag=tag + "xn")
        nc.scalar.activation(out=xn, in_=src_ap, func=AF.Identity,
                             bias=negmr, scale=rstd)
        return xn

    with tile.TileContext(nc) as tc:
        with (
            tc.tile_pool(name="consts", bufs=1) as consts,
            tc.tile_pool(name="persistA", bufs=1) as persistA,
        ):
            ctxT = persistA.tile([P, KH, SQ], f32r)      # 16 KB/part

            ident_b = consts.tile([P, P], bf16)
            make_identity(nc, ident_b)
            eps_t = consts.tile([P, 1], f32)
            nc.vector.memset(eps_t, EPS)
            negone = consts.tile([P, 1], f32)
            nc.vector.memset(negone, -1.0)
            ident_f = consts.tile([P, P], f32)
            make_identity(nc, ident_f)
            ident_r = consts.tile([P, P], f32r)
            nc.scalar.copy(out=ident_r, in_=ident_f)
            # f32r constant 1.0 (memset can't emit f32r; ACT Copy(0*x+1) can;
            # the input is ignored at scale=0)
            ones_t = consts.tile([1, HD], f32r)
            nc.scalar.activation(out=ones_t, in_=ident_f[0:1, 0:HD], func=AF.Copy,
                                 bias=1.0, scale=0.0)

            # persistB (phase-3+ accumulators) and the prefetch pools open
            # BEFORE phase 1 so they get SBUF space disjoint from the
            # phase-1/2 pools: their SP-queue DMAs (emitted after the
            # phase-1 loop) then stream during attention with no space-reuse
            # dependencies. prefetch_es closes right after phase 3 (LIFO) to
            # make room for the FFN pools.
            persistB_es = ExitStack()
            persistB = persistB_es.enter_context(tc.tile_pool(name="persistB", bufs=1))
            out_res = persistB.tile([P, TT_Q, H], f32)   # 16 KB/part
            stats2 = persistB.tile([P, TT_Q, KH, 6], f32)

            w1p_es = ExitStack()
            w1p = w1p_es.enter_context(tc.tile_pool(name="w1p", bufs=4))

            prefetch_es = ExitStack()
            wop = prefetch_es.enter_context(tc.tile_pool(name="wo", bufs=1))
            wqkv = prefetch_es.enter_context(tc.tile_pool(name="wqkv", bufs=2))
            qtp = prefetch_es.enter_context(tc.tile_pool(name="qt", bufs=2))

            with tc.tile_pool(name="xnTp", bufs=1) as xnTp:
                xnTc = [xnTp.tile([P, KH, SQ], bf16, tag=f"xnt{c}", name=f"xnt{c}")
                        for c in range(4)]               # 4 x 8 KB/part

                with tc.tile_pool(name="vap", bufs=1) as vap:
                    # token-major V (+ ones column for softmax denominator)
                    va = vap.tile([P, TT_S, NH, HD + 1], bf16)
                    nc.vector.memset(va[:, :, :, HD:HD + 1], 1.0)

                    # ------- Phase 1: LN1 + transpose + V projection -------
                    with (
                        tc.tile_pool(name="ln", bufs=4) as ln,
                        tc.tile_pool(name="lnst", bufs=4) as lnst,
                        tc.tile_pool(name="wvp", bufs=1) as wvp,
                        tc.tile_pool(name="ps_t", bufs=3, space="PSUM") as ps_t,
                        tc.tile_pool(name="ps_v", bufs=3, space="PSUM") as ps_v,
                    ):
                        # LN is software-pipelined two tiles ahead of the
                        # transpose/V work so PE never waits on the LN chain.
                        xns = {}

                        def emit_ln(i):
                            xt = ln.tile([P, H], f32, tag="xt", name="xt")
                            if i < 3:
                                # halves so LN stats start ~1.5us sooner on
                                # the pipeline-fill tiles
                                nc.sync.dma_start(out=xt[:, 0:512],
                                                  in_=xp[i * P:(i + 1) * P, 0:512])
                                nc.sync.dma_start(out=xt[:, 512:1024],
                                                  in_=xp[i * P:(i + 1) * P, 512:1024])
                            else:
                                nc.sync.dma_start(out=xt, in_=xp[i * P:(i + 1) * P, :])
                            xns[i] = layernorm_tile(lnst, xt, "l1", xn_dt=bf16, tc=tc)

                        emit_ln(0)
                        emit_ln(1)
                        emit_ln(2)

                        # Wv quarters queue behind the first x tiles on the
                        # (serial) DMA pipe.
                        wv = wvp.tile([P, KH, H], bf16)
                        wv_view = Wv.rearrange("(a p) c -> p a c", p=P)
                        nc.scalar.dma_start(out=wv[:, :, 0:256], in_=wv_view[:, :, 0:256])
                        nc.gpsimd.dma_start(out=wv[:, :, 256:512], in_=wv_view[:, :, 256:512])
                        nc.scalar.dma_start(out=wv[:, :, 512:768], in_=wv_view[:, :, 512:768])
                        nc.gpsimd.dma_start(out=wv[:, :, 768:1024], in_=wv_view[:, :, 768:1024])

                        bq_t = consts.tile([P, KH], f32)
                        bk_t = consts.tile([P, KH], f32)
                        bo2_t = consts.tile([P, KH], f32)
                        bf2_t = consts.tile([P, KH], f32)
                        bf1_t = consts.tile([P, FF // P], f32)
                        nc.gpsimd.dma_start(out=bq_t, in_=col_view(bq, KH))
                        nc.gpsimd.dma_start(out=bk_t, in_=col_view(bk, KH))
                        nc.gpsimd.dma_start(out=bo2_t, in_=col_view(bo2, KH))
                        nc.gpsimd.dma_start(out=bf2_t, in_=col_view(bf2, KH))
                        nc.gpsimd.dma_start(out=bf1_t, in_=col_view(bf1, FF // P))

                        def emit_v(i):
                            # V projection for tile i (runs one tile behind
                            # the transposes so PE never waits on the
                            # PSUM->SBUF evacuation of xnT)
                            for fg in range(2):
                                psw = ps_v.tile([P, SQ], f32, tag="psv",
                                                name="psw")
                                for ks in range(KH):
                                    nc.tensor.matmul(
                                        psw, xnTc[i // 4][:, ks, (i % 4) * P:(i % 4 + 1) * P],
                                        wv[:, ks, fg * SQ:(fg + 1) * SQ],
                                        start=(ks == 0), stop=(ks == KH - 1))
                                dstv = va[:, i, 8 * fg:8 * fg + 8, 0:HD]
                                src = psw.rearrange("p (h d) -> p h d", d=HD)
                                if fg == 0:
                                    nc.vector.tensor_copy(out=dstv, in_=src)
                                else:
                                    nc.scalar.copy(out=dstv, in_=src)

                        def emit_q0():
                            # Q projection for head-group 0 (only needs
                            # xnTc[0], ready after tile 3) — fills the PE
                            # starvation window while LN1 still paces tiles.
                            wk4 = wqkv.tile([P, KH, 256], bf16, tag="wk4",
                                            name="wk4")
                            wq4 = wqkv.tile([P, KH, 256], bf16, tag="wq4",
                                            name="wq4")
                            nc.gpsimd.dma_start(
                                out=wk4,
                                in_=Wk.rearrange("(a p) c -> p a c", p=P)[:, :, 0:256])
                            nc.gpsimd.dma_start(
                                out=wq4,
                                in_=Wq.rearrange("(a p) c -> p a c", p=P)[:, :, 0:256])
                            QTp = []
                            for pair in range(2):
                                ps = ps_v.tile([P, SQ], f32, tag="psv",
                                               name="psq0")
                                for ks in range(KH):
                                    nc.tensor.matmul(
                                        ps, wq4[:, ks, pair * P:(pair + 1) * P],
                                        xnTc[0][:, ks, :],
                                        start=(ks == 0), stop=(ks == KH - 1))
                                qt0 = qtp.tile([P, SQ], bf16, tag=f"qtp{pair}",
                                               name=f"qtp{pair}")
                                nc.vector.tensor_scalar_add(
                                    out=qt0, in0=ps, scalar1=bq_t[:, pair:pair + 1])
                                QTp.append(qt0)
                            return wk4, wq4, QTp


                        g0_hoist = {}
                        for i in range(TT_S):
                            if i + 3 < TT_S:
                                emit_ln(i + 3)
                            if i == 4:
                                g0_hoist["v"] = emit_q0()
                            xn = xns.pop(i)
                            for j2 in range(2):
                                ps = ps_t.tile([P, 4, P], bf16, tag="pst")
                                for k in range(4):
                                    j = 4 * j2 + k
                                    nc.tensor.transpose(
                                        ps[:, k, :], xn[:, j * P:(j + 1) * P], ident_b)
                                dst = xnTc[i // 4][:, 4 * j2:4 * j2 + 4,
                                                  (i % 4) * P:(i % 4 + 1) * P]
                                if j2 == 0:
                                    nc.vector.tensor_copy(out=dst, in_=ps)
                                else:
                                    nc.scalar.copy(out=dst, in_=ps)
                            if i >= 1:
                                emit_v(i - 1)
                        emit_v(TT_S - 1)

                    # Prefetch DMAs for phase 3+ (SP queue drains these
                    # during attention; pools were opened before phase 1).
                    wo_ts = []
                    for os_ in range(KH):
                        wo_t = wop.tile([P, KH, P], f32r, tag=f"wo{os_}",
                                        name=f"wo{os_}")
                        nc.sync.dma_start(
                            out=wo_t,
                            in_=Wo.rearrange("(a p) c -> p a c", p=P)[:, :, os_ * P:(os_ + 1) * P].bitcast(f32r))
                        wo_ts.append(wo_t)
                    w1_ts = {}
                    for ft in range(4):
                        w1_t = w1p.tile([P, KH, P], f32r, tag="w1_t", name="w1_t")
                        nc.sync.dma_start(
                            out=w1_t,
                            in_=W1.rearrange("(a p) c -> p a c", p=P)[:, :, ft * P:(ft + 1) * P].bitcast(f32r))
                        w1_ts[ft] = w1_t

                    # ---------------- Phase 2: K/Q + attention ----------------
                    with ExitStack() as es2:
                        ktp = es2.enter_context(tc.tile_pool(name="kt", bufs=2))
                        pexp = es2.enter_context(tc.tile_pool(name="pexp", bufs=6))
                        attn_sm = es2.enter_context(tc.tile_pool(name="attn_sm", bufs=2))
                        ps_qkv = es2.enter_context(tc.tile_pool(name="ps_qkv", bufs=2, space="PSUM"))
                        ps_s = es2.enter_context(tc.tile_pool(name="ps_s", bufs=3, space="PSUM"))
                        ps_ctx = es2.enter_context(tc.tile_pool(name="ps_ctx", bufs=1, space="PSUM"))
                        ps_b = es2.enter_context(tc.tile_pool(name="ps_b", bufs=1, space="PSUM"))
                        for g4 in range(4):
                            c0 = g4 * 4 * HD  # first feature column of this group
                            if g4 == 0:
                                wk4, wq4, QTp = g0_hoist.pop("v")
                            else:
                                wk4 = wqkv.tile([P, KH, 256], bf16, tag="wk4")
                                wq4 = wqkv.tile([P, KH, 256], bf16, tag="wq4")
                                nc.gpsimd.dma_start(
                                    out=wk4,
                                    in_=Wk.rearrange("(a p) c -> p a c", p=P)[:, :, c0:c0 + 256])
                                nc.gpsimd.dma_start(
                                    out=wq4,
                                    in_=Wq.rearrange("(a p) c -> p a c", p=P)[:, :, c0:c0 + 256])

                                # Q^T pair-stacked: [128(2 heads), SQ] per pair
                                QTp = [qtp.tile([P, SQ], bf16, tag=f"qtp{pair}",
                                                name=f"qtp{pair}") for pair in range(2)]
                                for pair in range(2):
                                    hp = 2 * g4 + pair
                                    ps = ps_qkv.tile([P, SQ], f32, tag="psqkv")
                                    for ks in range(KH):
                                        nc.tensor.matmul(
                                            ps, wq4[:, ks, pair * P:(pair + 1) * P],
                                            xnTc[0][:, ks, :],
                                            start=(ks == 0), stop=(ks == KH - 1))
                                    nc.vector.tensor_scalar_add(
                                        out=QTp[pair], in0=ps,
                                        scalar1=bq_t[:, hp:hp + 1])

                            for pair in range(2):
                                hp = 2 * g4 + pair
                                # K^T pair-stacked: [128(2 heads), S]
                                KTp = ktp.tile([P, S], bf16, tag="ktp")
                                for t4 in range(4):
                                    ps = ps_qkv.tile([P, SQ], f32, tag="psqkv")
                                    for ks in range(KH):
                                        nc.tensor.matmul(
                                            ps, wk4[:, ks, pair * P:(pair + 1) * P],
                                            xnTc[t4][:, ks, :],
                                            start=(ks == 0), stop=(ks == KH - 1))
                                    nc.vector.tensor_scalar_add(
                                        out=KTp[:, t4 * SQ:(t4 + 1) * SQ], in0=ps,
                                        scalar1=bk_t[:, hp:hp + 1])

                                # both heads of the pair interleaved per key
                                # chunk (disjoint PE row groups 0/64).
                                cpss = [ps_ctx.tile([HD + 1, SQ], f32,
                                                    tag=f"ctxps{rh}",
                                                    name=f"ctxps{rh}")
                                        for rh in range(2)]
                                for kc in range(TT_S):
                                    pts = []
                                    for rh in range(2):
                                        rb = rh * HD
                                        sps = ps_s.tile([P, SQ], f32, tag="sps")
                                        nc.tensor.matmul(
                                            sps, KTp[rb:rb + HD, kc * P:(kc + 1) * P],
                                            QTp[pair][rb:rb + HD, :],
                                            start=True, stop=True)
                                        pt = pexp.tile([P, SQ], bf16, tag="pt")
                                        nc.scalar.activation(
                                            out=pt, in_=sps, func=AF.Exp,
                                            scale=float(1.0 / np.sqrt(HD)))
                                        pts.append(pt)
                                    for rh in range(2):
                                        hh = 2 * pair + rh
                                        nc.tensor.matmul(
                                            cpss[rh], va[:, kc, 4 * g4 + hh, :], pts[rh],
                                            start=(kc == 0), stop=(kc == TT_S - 1))
                                for rh in range(2):
                                    hh = 2 * pair + rh   # head within group
                                    h = 4 * g4 + hh      # global head
                                    cps = cpss[rh]
                                    # ctx[0:64,:] / l  (l = cps[64,:])
                                    rl = attn_sm.tile([1, SQ], f32r, tag="rl")
                                    with nc.allow_low_precision(reason="softmax denom bcast"):
                                        nc.vector.reciprocal(out=rl, in_=cps[HD:HD + 1, :])
                                    # PE broadcast of 1/l to 64 rows
                                    bps = ps_b.tile([HD, SQ], f32, tag="bps")
                                    nc.tensor.matmul(bps, ones_t, rl,
                                                     start=True, stop=True)
                                    bsb = attn_sm.tile([HD, SQ], f32, tag="bsb")
                                    nc.vector.tensor_copy(out=bsb, in_=bps)
                                    slot = h // 2
                                    dst = ctxT[rh * HD:(rh + 1) * HD, slot, :]
                                    nc.vector.tensor_mul(dst, cps[0:HD, :], bsb)

            # xnT freed here.
            if True:
                # ---------------- Phase 3: output proj + residual ----------
                with (
                    tc.tile_pool(name="xrp", bufs=1) as xrp,
                    tc.tile_pool(name="ot", bufs=3) as otp,
                    tc.tile_pool(name="ps_o", bufs=3, space="PSUM") as ps_o,
                    tc.tile_pool(name="ps_t3", bufs=2, space="PSUM") as ps_t3,
                ):
                    xr = xrp.tile([P, TT_Q, H], f32)
                    for tt in range(TT_Q):
                        nc.sync.dma_start(out=xr[:, tt, :],
                                          in_=xp[tt * P:(tt + 1) * P, :])
                    oTs = {}

                    def emit_o_tail(os_):
                        # transposes + residual add + LN2 stats, one os_
                        # behind the matmuls so PE never waits on the ACT
                        # PSUM evacuation. The last os_ is latency-critical
                        # (it gates the whole LN2->FFN chain), so its add is
                        # split per tile to overlap with the stats.
                        oT = oTs.pop(os_)
                        pst = ps_t3.tile([P, TT_Q, P], f32r, tag="pst3")
                        for tt in range(TT_Q):
                            nc.tensor.transpose(pst[:, tt, :],
                                                oT[:, tt * P:(tt + 1) * P], ident_r)
                        nc.vector.tensor_add(
                            out_res[:, :, os_ * P:(os_ + 1) * P], pst,
                            xr[:, :, os_ * P:(os_ + 1) * P])
                        for tt in range(TT_Q):
                            nc.vector.bn_stats(
                                out=stats2[:, tt, os_, :],
                                in_=out_res[:, tt, os_ * P:(os_ + 1) * P])

                    for os_ in range(KH):
                        ps = ps_o.tile([P, SQ], f32, tag="pso")
                        for cs in range(KH):
                            nc.tensor.matmul(ps, wo_ts[os_][:, cs, :], ctxT[:, cs, :],
                                             start=(cs == 0), stop=(cs == KH - 1))
                        oT = otp.tile([P, SQ], f32r, tag="oT")
                        nc.scalar.activation(out=oT, in_=ps, func=AF.Identity,
                                             bias=bo2_t[:, os_:os_ + 1], scale=1.0)
                        oTs[os_] = oT
                        if os_ >= 1:
                            emit_o_tail(os_ - 1)
                    emit_o_tail(KH - 1)

                # release the wo/wqkv prefetch space (LIFO top); xn2T takes
                # that space for the FFN.
                prefetch_es.close()
                w1xp_es = ExitStack()
                w1xp = w1xp_es.enter_context(tc.tile_pool(name="w1xp", bufs=1))
                xn2T = w1xp.tile([P, KH, SQ], f32r)          # 16 KB/part

                # ---------------- Phase 4: LN2 -> xn2^T ----------------
                with (
                    tc.tile_pool(name="lnst2", bufs=4) as lnst2,
                    tc.tile_pool(name="ps_t4", bufs=2, space="PSUM") as ps_t4,
                ):
                    # batched scalar chain for all 4 tiles (one hop each),
                    # high priority: this latency gates the whole FFN
                    mv4 = lnst2.tile([P, TT_Q, 2], f32)
                    rstd4 = lnst2.tile([P, TT_Q], f32)
                    negmr4 = lnst2.tile([P, TT_Q], f32)
                    with tc.high_priority():
                        for i in range(TT_Q):
                            nc.vector.bn_aggr(out=mv4[:, i, :], in_=stats2[:, i, :, :])
                        nc.scalar.activation(out=rstd4, in_=mv4[:, :, 1], func=AF.Sqrt,
                                             bias=eps_t, scale=1.0)
                        nc.vector.reciprocal(out=rstd4, in_=rstd4)
                        nc.vector.scalar_tensor_tensor(
                            out=negmr4, in0=mv4[:, :, 0], scalar=negone, in1=rstd4,
                            op0=ALU.mult, op1=ALU.mult)
                    for i in range(TT_Q):
                        xn2 = lnst2.tile([P, H], f32r, tag="xn2")
                        for hh2 in range(2):
                            nc.scalar.activation(
                                out=xn2[:, hh2 * 512:(hh2 + 1) * 512],
                                in_=out_res[:, i, hh2 * 512:(hh2 + 1) * 512],
                                func=AF.Identity,
                                bias=negmr4[:, i:i + 1], scale=rstd4[:, i:i + 1])
                        for j2 in range(2):
                            ps = ps_t4.tile([P, 4, P], f32r, tag="pst4")
                            for k in range(4):
                                j = 4 * j2 + k
                                nc.tensor.transpose(
                                    ps[:, k, :], xn2[:, j * P:(j + 1) * P], ident_r)
                            dst = xn2T[:, 4 * j2:4 * j2 + 4, i * P:(i + 1) * P]
                            nc.vector.tensor_copy(out=dst, in_=ps)

                # ---------------- Phase 5: FFN ----------------
                with tc.tile_pool(name="hTp", bufs=1) as hTp:
                    hT = hTp.tile([P, FF // P, SQ], f32r)   # 64 KB/part
                    with (
                        tc.tile_pool(name="w2p", bufs=6) as w2p,
                        tc.tile_pool(name="ps_f", bufs=4, space="PSUM") as ps_f,
                    ):
                        for ft in range(FF // P):
                            if ft in w1_ts:
                                w1_t = w1_ts.pop(ft)
                            else:
                                w1_t = w1p.tile([P, KH, P], f32r, tag="w1_t",
                                                name="w1_t")
                                nc.sync.dma_start(
                                    out=w1_t,
                                    in_=W1.rearrange("(a p) c -> p a c", p=P)[:, :, ft * P:(ft + 1) * P].bitcast(f32r))
                            ps = ps_f.tile([P, SQ], f32, tag="psf")
                            for ks in range(KH):
                                nc.tensor.matmul(ps, w1_t[:, ks, :], xn2T[:, ks, :],
                                                 start=(ks == 0), stop=(ks == KH - 1))
                            nc.scalar.activation(out=hT[:, ft, :], in_=ps, func=AF.Relu,
                                                 bias=bf1_t[:, ft:ft + 1], scale=1.0)

                        with (
                            tc.tile_pool(name="o2", bufs=3) as o2p,
                            tc.tile_pool(name="yout", bufs=2) as youtp,
                            tc.tile_pool(name="ps_o2", bufs=2, space="PSUM") as ps_o2,
                            tc.tile_pool(name="ps_t5", bufs=2, space="PSUM") as ps_t5,
                        ):
                            o2Ts = {}

                            def emit_y_tail(os_):
                                o2T = o2Ts.pop(os_)
                                pst = ps_t5.tile([P, TT_Q, P], f32r, tag="pst5")
                                for tt in range(TT_Q):
                                    nc.tensor.transpose(pst[:, tt, :],
                                                        o2T[:, tt * P:(tt + 1) * P], ident_r)
                                yo = youtp.tile([P, TT_Q, P], f32, tag="yo")
                                nc.vector.tensor_add(
                                    yo, pst, out_res[:, :, os_ * P:(os_ + 1) * P])
                                nc.sync.dma_start(
                                    out=y.rearrange("(t p) c -> p t c", p=P)[:, :, os_ * P:(os_ + 1) * P],
                                    in_=yo)

                            for os_ in range(KH):
                                ps = ps_o2.tile([P, SQ], f32, tag="pso2")
                                for q4 in range(4):
                                    w2_t = w2p.tile([P, 8, P], f32r, tag="w2_t")
                                    dma_eng = nc.gpsimd if q4 % 2 == 0 else nc.sync
                                    dma_eng.dma_start(
                                        out=w2_t,
                                        in_=W2.rearrange("(a p) c -> p a c", p=P)[:, q4 * 8:(q4 + 1) * 8, os_ * P:(os_ + 1) * P].bitcast(f32r))
                                    for f8 in range(8):
                                        ft = q4 * 8 + f8
                                        nc.tensor.matmul(ps, w2_t[:, f8, :], hT[:, ft, :],
                                                         start=(ft == 0), stop=(ft == FF // P - 1))
                                o2T = o2p.tile([P, SQ], f32r, tag="o2T")
                                nc.scalar.activation(out=o2T, in_=ps, func=AF.Identity,
                                                     bias=bf2_t[:, os_:os_ + 1], scale=1.0)
                                o2Ts[os_] = o2T
                                if os_ >= 1:
                                    emit_y_tail(os_ - 1)
                            emit_y_tail(KH - 1)

                # LIFO pool teardown for the manually-entered pools
                w1xp_es.close()
                prefetch_es2 = None  # (placeholder, nothing else open here)
                w1p_es.close()
                persistB_es.close()

    nc.finalize()
    return nc


_NC_CACHE = {}


def _prepare_in_maps(inputs):
    """Host-side weight algebra + per-core sharding (see module docstring).
    Returns the per-core input maps for run_bass_kernel_spmd."""
    import ml_dtypes

    x = np.ascontiguousarray(np.asarray(inputs["x"], dtype=np.float32))
    f64 = np.float64
    g1 = np.asarray(inputs["g1"], f64)
    b1 = np.asarray(inputs["b1"], f64)
    g2 = np.asarray(inputs["g2"], f64)
    b2 = np.asarray(inputs["b2"], f64)
    Wq = np.asarray(inputs["Wq"], f64)
    Wk = np.asarray(inputs["Wk"], f64)
    Wv = np.asarray(inputs["Wv"], f64)
    Wo = np.asarray(inputs["Wo"], f64)
    W1 = np.asarray(inputs["W1"], f64)

    bf16 = ml_dtypes.bfloat16
    # exact affine folds (see module docstring)
    Wq_f = (g1[:, None] * Wq).astype(np.float32).astype(bf16)
    Wk_f = (g1[:, None] * Wk).astype(np.float32).astype(bf16)
    Wv_f = (g1[:, None] * Wv).astype(np.float32).astype(bf16)
    W1_f = (g2[:, None] * W1).astype(np.float32)
    bq_f = (np.asarray(inputs["bq"], f64) + b1 @ Wq).astype(np.float32)
    bk_f = (np.asarray(inputs["bk"], f64) + b1 @ Wk).astype(np.float32)
    bv_f = np.asarray(inputs["bv"], f64) + b1 @ Wv
    bo2 = (np.asarray(inputs["bo"], f64) + bv_f @ Wo).astype(np.float32)
    bf1_f = (np.asarray(inputs["bf1"], f64) + b2 @ W1).astype(np.float32)

    weights = {
        "Wq": np.ascontiguousarray(Wq_f), "Wk": np.ascontiguousarray(Wk_f),
        "Wv": np.ascontiguousarray(Wv_f),
        "Wo": np.ascontiguousarray(np.asarray(inputs["Wo"], np.float32)),
        "W1": np.ascontiguousarray(W1_f),
        "W2": np.ascontiguousarray(np.asarray(inputs["W2"], np.float32)),
        "bq": bq_f, "bk": bk_f, "bo2": bo2, "bf1": bf1_f,
        "bf2": np.ascontiguousarray(np.asarray(inputs["bf2"], np.float32)),
    }

    in_maps = []
    for c in range(N_CORES):
        b, qc = c // 4, c % 4
        xb = np.roll(x[b], -qc * SQ, axis=0)
        m = {"xp": np.ascontiguousarray(xb)}
        m.update(weights)
        in_maps.append(m)
    return in_maps


def kernel(**inputs):
    import concourse.bass_utils as bass_utils

    if "nc" not in _NC_CACHE:
        _NC_CACHE["nc"] = _build_nc()
    nc = _NC_CACHE["nc"]
    in_maps = _prepare_in_maps(inputs)
    res = bass_utils.run_bass_kernel_spmd(nc, in_maps, core_ids=list(range(N_CORES)))
    out = np.empty((B, S, H), dtype=np.float32)
    for c in range(N_CORES):
        b, qc = c // 4, c % 4
        out[b, qc * SQ:(qc + 1) * SQ, :] = res.results[c]["y"]
    return out
